# revision 9
# baseline (speedup 1.0000x reference)
"""Trainium2 Bass kernel for nn_Decoder: 3-layer LSTM decoder with attention.

Strategy: tensor-parallel over the hidden/gate dim across 8 cores (each core
holds a 512-row gate slice of every LSTM cell, fp32, resident in SBUF),
with 4 small AllGathers per timestep (h1,h2,h3,ctx — all sent transposed so
the gathered result is directly usable as matmul lhsT chunks). Attention is
batch-sharded (8 batch elements per core, keys/values bf16 in SBUF). The
final score projection is hoisted out of the time loop and computed as one
batched matmul at the end. Softmax uses sigmoid (exp(x) = s/(1-s)) so the
whole kernel uses one ACT table set.
"""
import numpy as np
import concourse.bass as bass
import concourse.bacc as bacc
import concourse.mybir as mybir
import concourse.tile as tile
from concourse.bass_utils import run_bass_kernel_spmd

N_CORES = 8
B, S, E, H, K, V = 64, 400, 512, 1024, 512, 34
BL = B // N_CORES       # 8 local batch elements (attention)
GS = 4 * H // N_CORES   # 512 gate rows per core
HS = H // N_CORES       # 128 h-units per core
F32 = mybir.dt.float32
F16 = mybir.dt.float16
SIG = mybir.ActivationFunctionType.Sigmoid
TANH = mybir.ActivationFunctionType.Tanh
COPY = mybir.ActivationFunctionType.Copy
AX = mybir.AxisListType.X
MUL = mybir.AluOpType.mult
ADD = mybir.AluOpType.add

_cache = {}


def _build(T):
    nc = bacc.Bacc("TRN2", target_bir_lowering=False, debug=False,
                   enable_asserts=False, num_devices=N_CORES)
    dt = nc.dram_tensor
    # per-core weight shards (transposed, chunked on contract dim)
    wctx_d = dt("wctx", [4, 128, GS], F16, kind="ExternalInput")
    whh0_d = dt("whh0", [8, 128, GS], F16, kind="ExternalInput")
    wih1_d = dt("wih1", [8, 128, GS], F16, kind="ExternalInput")
    whh1_d = dt("whh1", [8, 128, GS], F16, kind="ExternalInput")
    wih2_d = dt("wih2", [8, 128, GS], F16, kind="ExternalInput")
    whh2_d = dt("whh2", [8, 128, GS], F16, kind="ExternalInput")
    wproj_d = dt("wproj", [8, 128, K], F16, kind="ExternalInput")
    wscore_d = dt("wscore", [8, 128, V], F32, kind="ExternalInput")
    embg_d = dt("embg", [V + 1, GS], F16, kind="ExternalInput")
    bias1_d = dt("bias1", [1, GS], F16, kind="ExternalInput")
    bias2_d = dt("bias2", [1, GS], F16, kind="ExternalInput")
    bproj_d = dt("bproj", [1, K], F16, kind="ExternalInput")
    bscore_d = dt("bscore", [V, 1], F32, kind="ExternalInput")
    oh_d = dt("oh", [T, V + 1, B], F16, kind="ExternalInput")
    sel_d = dt("sel", [B, BL], F32, kind="ExternalInput")
    eye64_d = dt("eye64", [B, B], F32, kind="ExternalInput")
    eye8_d = dt("eye8", [BL, BL], F32, kind="ExternalInput")
    keys_d = dt("keys_t", [BL, 4, 128, S], F16, kind="ExternalInput")
    vals_d = dt("vals_t", [BL, 4, 128, K], F16, kind="ExternalInput")
    attn_o = dt("attn_out", [T, BL, S], F32, kind="ExternalOutput")
    score_o = dt("scores_out", [V, T * BL], F32, kind="ExternalOutput")

    with tile.TileContext(nc) as tc:
        with (
            tc.tile_pool(name="const", bufs=1) as cp,
            tc.tile_pool(name="work", bufs=2) as wp,
            tc.tile_pool(name="gath", bufs=2) as gp,
            tc.tile_pool(name="ps", bufs=1, space="PSUM") as pp,
            tc.tile_pool(name="dram", bufs=2, space="DRAM") as dp,
        ):
            # ---- resident tensors ----
            wctx = cp.tile([128, 4, GS], F16)
            nc.sync.dma_start(wctx[:], wctx_d.rearrange("c p g -> p c g"))
            whh0 = cp.tile([128, 8, GS], F16)
            nc.sync.dma_start(whh0[:], whh0_d.rearrange("c p g -> p c g"))
            wih1 = cp.tile([128, 8, GS], F16)
            nc.sync.dma_start(wih1[:], wih1_d.rearrange("c p g -> p c g"))
            whh1 = cp.tile([128, 8, GS], F16)
            nc.sync.dma_start(whh1[:], whh1_d.rearrange("c p g -> p c g"))
            wih2 = cp.tile([128, 8, GS], F16)
            nc.sync.dma_start(wih2[:], wih2_d.rearrange("c p g -> p c g"))
            whh2 = cp.tile([128, 8, GS], F16)
            nc.sync.dma_start(whh2[:], whh2_d.rearrange("c p g -> p c g"))
            wproj = cp.tile([128, 8, K], F16)
            nc.sync.dma_start(wproj[:], wproj_d.rearrange("c p g -> p c g"))
            wscore = cp.tile([128, 8, V], F32)
            nc.sync.dma_start(wscore[:], wscore_d.rearrange("c p g -> p c g"))
            embg = cp.tile([V + 1, GS], F16)
            nc.sync.dma_start(embg[:], embg_d[:])
            bias1 = cp.tile([1, GS], F16)
            nc.sync.dma_start(bias1[:], bias1_d[:])
            bias2 = cp.tile([1, GS], F16)
            nc.sync.dma_start(bias2[:], bias2_d[:])
            bproj = cp.tile([1, K], F16)
            nc.sync.dma_start(bproj[:], bproj_d[:])
            bscore = cp.tile([V, 1], F32)
            nc.sync.dma_start(bscore[:], bscore_d[:])
            sel = cp.tile([B, BL], F32)
            nc.sync.dma_start(sel[:], sel_d[:])
            eye64 = cp.tile([B, B], F32)
            nc.sync.dma_start(eye64[:], eye64_d[:])
            eye8 = cp.tile([BL, BL], F32)
            nc.sync.dma_start(eye8[:], eye8_d[:])
            keyst = cp.tile([128, BL, 4, S], F16)
            nc.sync.dma_start(keyst[:], keys_d.rearrange("b c p s -> p b c s"))
            valst = cp.tile([128, BL, 4, K], F16)
            nc.sync.dma_start(valst[:], vals_d.rearrange("b c p s -> p b c s"))
            ones = cp.tile([1, B], F16)
            nc.vector.memset(ones[:], 1.0)
            # LSTM cell state (updated in place each step)
            c1 = cp.tile([B, HS], F32)
            nc.vector.memset(c1[:], 0.0)
            c2 = cp.tile([B, HS], F32)
            nc.vector.memset(c2[:], 0.0)
            c3 = cp.tile([B, HS], F32)
            nc.vector.memset(c3[:], 0.0)
            # DRAM accumulators for deferred score matmul
            qT_acc = dp.tile([T, 128, 4 * BL], F32, bufs=1)
            cT_acc = dp.tile([T, 128, 4 * BL], F32, bufs=1)

            prev = {"h1T": None, "h2T": None, "h3T": None, "ctxT": None}

            def gather_hT(h, tag):
                """transpose h [64,HS] -> [HS,64], allgather -> [128, 8, 64]."""
                tp_ps = pp.tile([128, B], F32, tag="tp", bufs=2)
                nc.tensor.matmul(tp_ps[:], h[:], eye64[:], start=True, stop=True)
                hT = wp.tile([128, B], F16, tag=f"hT{tag}", bufs=2)
                nc.vector.tensor_copy(hT[:], tp_ps[:])
                bin_ = dp.tile([128, B], F16, tag=f"bi{tag}", name=f"bi{tag}")
                nc.sync.dma_start(bin_[:], hT[:])
                bout = dp.tile([N_CORES, 128, B], F16, addr_space="Shared",
                               tag=f"bo{tag}", name=f"bo{tag}")
                nc.gpsimd.collective_compute(
                    "AllGather", mybir.AluOpType.bypass,
                    replica_groups=[list(range(N_CORES))],
                    ins=[bin_.opt()], outs=[bout.opt()])
                hT_all = gp.tile([128, N_CORES, B], F16, tag=f"ga{tag}")
                nc.sync.dma_start(hT_all[:], bout.rearrange("c p b -> p c b"))
                return hT_all

            def cell_nl(g_ps, c_st, tag):
                """gates psum [64, GS] (i|f|o|g) + state -> h [64, HS]."""
                sfo = wp.tile([B, 3 * HS], F32, tag=f"sfo{tag}")
                nc.scalar.activation(sfo[:], g_ps[:, 0:3 * HS], SIG)
                tg = wp.tile([B, HS], F32, tag=f"tg{tag}")
                nc.scalar.activation(tg[:], g_ps[:, 3 * HS:GS], TANH)
                t1 = wp.tile([B, HS], F32, tag=f"t1{tag}")
                nc.vector.tensor_tensor(t1[:], sfo[:, HS:2 * HS], c_st[:], MUL)
                t2 = wp.tile([B, HS], F32, tag=f"t2{tag}")
                nc.vector.tensor_tensor(t2[:], sfo[:, 0:HS], tg[:], MUL)
                nc.vector.tensor_tensor(c_st[:], t1[:], t2[:], ADD)
                tc_ = wp.tile([B, HS], F32, tag=f"tc{tag}")
                nc.scalar.activation(tc_[:], c_st[:], TANH)
                h = wp.tile([B, HS], F32, tag=f"h{tag}", bufs=2)
                nc.vector.tensor_tensor(h[:], sfo[:, 2 * HS:3 * HS], tc_[:], MUL)
                return h

            for t in range(T):
                # ---- cell 0: emb one-hot + hh0(h1[t-1]) + ctx(t-1) ----
                oh_t = wp.tile([V + 1, B], F16, tag="oh")
                nc.sync.dma_start(oh_t[:], oh_d[t])
                mms = [(oh_t[:], embg[:])]
                if t > 0:
                    for c in range(8):
                        mms.append((prev["h1T"][:, c, :], whh0[:, c, :]))
                    for c in range(4):
                        mms.append((prev["ctxT"][:, c, :], wctx[:, c, :]))
                g0 = pp.tile([B, GS], F32, tag="gates", bufs=2)
                for i, (l, r) in enumerate(mms):
                    nc.tensor.matmul(g0[:], l, r, start=(i == 0),
                                     stop=(i == len(mms) - 1))
                h1 = cell_nl(g0, c1, "a")
                h1T_all = gather_hT(h1, "h1")

                # ---- cell 1 ----
                mms = [(ones[:], bias1[:])]
                if t > 0:
                    for c in range(8):
                        mms.append((prev["h2T"][:, c, :], whh1[:, c, :]))
                for c in range(8):
                    mms.append((h1T_all[:, c, :], wih1[:, c, :]))
                g1 = pp.tile([B, GS], F32, tag="gates", bufs=2)
                for i, (l, r) in enumerate(mms):
                    nc.tensor.matmul(g1[:], l, r, start=(i == 0),
                                     stop=(i == len(mms) - 1))
                h2 = cell_nl(g1, c2, "b")
                h2T_all = gather_hT(h2, "h2")

                # ---- cell 2 ----
                mms = [(ones[:], bias2[:])]
                if t > 0:
                    for c in range(8):
                        mms.append((prev["h3T"][:, c, :], whh2[:, c, :]))
                for c in range(8):
                    mms.append((h2T_all[:, c, :], wih2[:, c, :]))
                g2 = pp.tile([B, GS], F32, tag="gates", bufs=2)
                for i, (l, r) in enumerate(mms):
                    nc.tensor.matmul(g2[:], l, r, start=(i == 0),
                                     stop=(i == len(mms) - 1))
                h3 = cell_nl(g2, c3, "c")
                h3T_all = gather_hT(h3, "h3")

                # ---- q = h3 @ w_proj.T + b_proj (replicated) ----
                q_ps = pp.tile([B, K], F32, tag="gates", bufs=2)
                nc.tensor.matmul(q_ps[:], ones[:], bproj[:], start=True, stop=False)
                for c in range(8):
                    nc.tensor.matmul(q_ps[:], h3T_all[:, c, :], wproj[:, c, :],
                                     start=False, stop=(c == 7))
                q_sb = wp.tile([B, K], F32, tag="qsb")
                nc.scalar.activation(q_sb[:], q_ps[:], COPY)
                # select my 8 batches + transpose: qT [128, 4, 8]
                qt_ps = pp.tile([128, 4 * BL], F32, tag="tp", bufs=2)
                for c in range(4):
                    nc.tensor.matmul(qt_ps[:, c * BL:(c + 1) * BL],
                                     q_sb[:, c * 128:(c + 1) * 128], sel[:],
                                     start=True, stop=True)
                qT_bf = wp.tile([128, 4 * BL], F16, tag="qTbf")
                nc.vector.tensor_copy(qT_bf[:], qt_ps[:])
                qT_f = wp.tile([128, 4 * BL], F32, tag="qTf")
                nc.vector.tensor_copy(qT_f[:], qt_ps[:])
                nc.sync.dma_start(qT_acc[t], qT_f[:])

                # ---- attention energies (per-b matvec, col-tiled 4x) ----
                e_ps = [pp.tile([128, S], F32, tag="en", bufs=2, name=f"e{g}_{t}")
                        for g in range(2)]
                for b in range(BL):
                    ps = e_ps[b // 4]
                    row = 32 * (b % 4)
                    for c in range(4):
                        nc.tensor.matmul(
                            ps[row:row + 1, :],
                            qT_bf[:, c * BL + b:c * BL + b + 1],
                            keyst[:, b, c, :],
                            start=(c == 0), stop=(c == 3),
                            tile_position=(0, row))
                # ---- compact energies into [8, S], then sigmoid-softmax ----
                e_sb = wp.tile([128, 2, S], F32, tag="esb", bufs=1)
                for b in range(BL):
                    g, row = b // 4, 32 * (b % 4)
                    nc.vector.tensor_copy(e_sb[row:row + 1, g, :],
                                          e_ps[g][row:row + 1, :])
                e8 = wp.tile([BL, S], F32, tag="e8")
                for g in range(2):
                    ev = e_sb.rearrange("(a b) g s -> a b g s", b=32)[:, 0, g, :]
                    nc.sync.dma_start(e8[g * 4:(g + 1) * 4, :], ev)
                m8 = wp.tile([BL, 1], F32, tag="m8")
                nc.vector.reduce_max(m8[:], e8[:], axis=AX)
                nm8 = wp.tile([BL, 1], F32, tag="nm8")
                nc.vector.tensor_scalar_mul(nm8[:], m8[:], -1.0)
                sg8 = wp.tile([BL, S], F32, tag="sg8")
                nc.scalar.activation(sg8[:], e8[:], SIG, bias=nm8[:])
                u8 = wp.tile([BL, S], F32, tag="u8")
                nc.vector.tensor_scalar(u8[:], sg8[:], -1.0, 1.0, op0=MUL, op1=ADD)
                nc.vector.reciprocal(u8[:], u8[:])
                attn8 = wp.tile([BL, S], F32, tag="attn8")
                nc.vector.tensor_tensor(attn8[:], sg8[:], u8[:], MUL)  # exp(y)
                z8 = wp.tile([BL, 1], F32, tag="z8")
                nc.vector.reduce_sum(z8[:], attn8[:], axis=AX)
                nc.vector.reciprocal(z8[:], z8[:])
                nc.vector.tensor_scalar_mul(attn8[:], attn8[:], z8[:])
                nc.sync.dma_start(attn_o[t], attn8[:])

                # ---- transpose attn -> attnT [128, 4, 8] (bf16) ----
                at_ps = pp.tile([128, 4 * BL], F32, tag="tp", bufs=2)
                for c in range(4):
                    pr = 128 if c < 3 else S - 3 * 128
                    nc.tensor.matmul(at_ps[0:pr, c * BL:(c + 1) * BL],
                                     attn8[:, c * 128:c * 128 + pr], eye8[:],
                                     start=True, stop=True)
                aT_bf = wp.tile([128, 4 * BL], F16, tag="aTbf")
                nc.vector.memset(aT_bf[:], 0.0)
                nc.vector.tensor_copy(aT_bf[:, 0:3 * BL], at_ps[:, 0:3 * BL])
                nc.vector.tensor_copy(aT_bf[0:S - 3 * 128, 3 * BL:4 * BL],
                                      at_ps[0:S - 3 * 128, 3 * BL:4 * BL])

                # ---- ctx = attn @ V (per-b, col-tiled) ----
                c_ps = [pp.tile([128, K], F32, tag="ctx", bufs=2, name=f"c{g}_{t}")
                        for g in range(2)]
                for b in range(BL):
                    ps = c_ps[b // 4]
                    row = 32 * (b % 4)
                    for c in range(4):
                        nc.tensor.matmul(
                            ps[row:row + 1, :],
                            aT_bf[:, c * BL + b:c * BL + b + 1],
                            valst[:, b, c, :],
                            start=(c == 0), stop=(c == 3),
                            tile_position=(0, row))
                # compact ctx rows into [8, K]
                c_sb = wp.tile([128, 2, K], F32, tag="csb", bufs=1)
                for b in range(BL):
                    g, row = b // 4, 32 * (b % 4)
                    nc.vector.tensor_copy(c_sb[row:row + 1, g, :],
                                          c_ps[g][row:row + 1, :])
                ctx8 = wp.tile([BL, K], F32, tag="ctx8")
                for g in range(2):
                    cv = c_sb.rearrange("(a b) g s -> a b g s", b=32)[:, 0, g, :]
                    nc.sync.dma_start(ctx8[g * 4:(g + 1) * 4, :], cv)
                # transpose ctx -> ctxT [128, 4, 8]
                ct_ps = pp.tile([128, 4 * BL], F32, tag="tp", bufs=2)
                for c in range(4):
                    nc.tensor.matmul(ct_ps[:, c * BL:(c + 1) * BL],
                                     ctx8[:, c * 128:(c + 1) * 128], eye8[:],
                                     start=True, stop=True)
                cT_f = wp.tile([128, 4 * BL], F32, tag="cTf")
                nc.vector.tensor_copy(cT_f[:], ct_ps[:])
                nc.sync.dma_start(cT_acc[t], cT_f[:])
                cT_h = wp.tile([128, 4 * BL], F16, tag="cTh")
                nc.vector.tensor_copy(cT_h[:], ct_ps[:])
                # allgather ctxT
                cbin = dp.tile([128, 4 * BL], F16, tag="cbi", name=f"cbi_{t}")
                nc.sync.dma_start(cbin[:], cT_h[:])
                cbout = dp.tile([N_CORES, 128, 4 * BL], F16, addr_space="Shared",
                                tag="cbo", name=f"cbo_{t}")
                nc.gpsimd.collective_compute(
                    "AllGather", mybir.AluOpType.bypass,
                    replica_groups=[list(range(N_CORES))],
                    ins=[cbin.opt()], outs=[cbout.opt()])
                ctxT_all = gp.tile([128, 4, N_CORES, BL], F16, tag="gactx")
                nc.sync.dma_start(
                    ctxT_all[:],
                    cbout.rearrange("c p (k b) -> p k c b", b=BL))

                prev = {"h1T": h1T_all, "h2T": h2T_all, "h3T": h3T_all,
                        "ctxT": ctxT_all.rearrange("p k c b -> p k (c b)")}

            # ---- deferred scores: [V, T*BL] = wscore.T @ [q;ctx] + b ----
            sc_sb = wp.tile([V, T * BL], F32, tag="scsb", bufs=1)
            for hf in range(2):
                t0, t1_ = hf * (T // 2), (hf + 1) * (T // 2)
                n = (t1_ - t0) * BL
                s_ps = pp.tile([V, n], F32, tag="en", bufs=2, name=f"sps{hf}")
                pairs = [(si, c) for si in range(2) for c in range(4)]
                for idx, (si, c) in enumerate(pairs):
                    src = (qT_acc, cT_acc)[si]
                    rhs_t = wp.tile([128, t1_ - t0, BL], F32, tag="srhs")
                    nc.sync.dma_start(
                        rhs_t[:],
                        src[t0:t1_].rearrange("t p (k b) -> p k t b", b=BL)
                        [:, c, :, :])
                    nc.tensor.matmul(s_ps[:], wscore[:, c + 4 * si, :],
                                     rhs_t[:], start=(idx == 0),
                                     stop=(idx == len(pairs) - 1))
                nc.scalar.activation(sc_sb[:, t0 * BL:t1_ * BL], s_ps[:],
                                     mybir.ActivationFunctionType.Identity,
                                     bias=bscore[:])
            nc.sync.dma_start(score_o[:], sc_sb[:])

    nc.compile()
    return nc


def _prep_inputs(inputs, T):
    """Build per-core in_maps from full inputs."""
    f32 = np.float32
    labels = np.asarray(inputs["labels"]).astype(np.int64)
    keys = np.asarray(inputs["keys"], f32)
    values = np.asarray(inputs["values"], f32)
    emb = np.asarray(inputs["emb"], f32)
    w_proj = np.asarray(inputs["w_proj"], f32)
    b_proj = np.asarray(inputs["b_proj"], f32)
    w_score = np.asarray(inputs["w_score"], f32)
    b_score = np.asarray(inputs["b_score"], f32)
    h0 = np.asarray(inputs["h0"], f32)

    eye64 = np.eye(B, dtype=f32)
    eye8 = np.eye(BL, dtype=f32)
    oh = np.zeros((T, V + 1, B), f32)
    for t in range(T):
        oh[t, labels[:, t], np.arange(B)] = 1.0
    oh[0, V, :] = 1.0  # initial-context constant row

    # torch gate order i,f,g,o ; our section order i,f,o,g
    blk = {"i": 0, "f": 1, "g": 2, "o": 3}
    in_maps = []
    ctx0 = (h0[0] @ w_proj.T + b_proj).astype(f32)  # [K], h0 rows identical
    for j in range(N_CORES):
        rows = np.concatenate([
            np.arange(blk[g] * H + j * HS, blk[g] * H + (j + 1) * HS)
            for g in ("i", "f", "o", "g")])

        def shard(w):
            return np.ascontiguousarray(np.asarray(w, f32)[rows])

        wih0_s = shard(inputs["w_ih0"])      # [GS, E+K]
        whh0_s = shard(inputs["w_hh0"])      # [GS, H]
        b0 = shard(inputs["b_ih0"]) + shard(inputs["b_hh0"])
        embg = np.concatenate([
            emb @ wih0_s[:, :E].T + b0[None, :],
            (ctx0 @ wih0_s[:, E:].T + 0.0)[None, :]], axis=0).astype(f32)

        def chunked(wT, nch, width):
            return np.ascontiguousarray(wT.reshape(nch, 128, width))

        sel = np.zeros((B, BL), f32)
        sel[np.arange(j * BL, (j + 1) * BL), np.arange(BL)] = 1.0

        kb = np.zeros((BL, 4, 128, S), np.float16)
        vb = np.zeros((BL, 4, 128, K), np.float16)
        for i in range(BL):
            b = j * BL + i
            kb[i] = keys[:, b, :].T.reshape(4, 128, S).astype(np.float16)
            vpad = np.zeros((512, K), f32)
            vpad[:S] = values[:, b, :]
            vb[i] = vpad.reshape(4, 128, K).astype(np.float16)

        in_maps.append({
            "wctx": chunked(np.ascontiguousarray(wih0_s[:, E:].T), 4, GS).astype(np.float16),
            "whh0": chunked(np.ascontiguousarray(whh0_s.T), 8, GS).astype(np.float16),
            "wih1": chunked(np.ascontiguousarray(shard(inputs["w_ih1"]).T), 8, GS).astype(np.float16),
            "whh1": chunked(np.ascontiguousarray(shard(inputs["w_hh1"]).T), 8, GS).astype(np.float16),
            "wih2": chunked(np.ascontiguousarray(shard(inputs["w_ih2"]).T), 8, GS).astype(np.float16),
            "whh2": chunked(np.ascontiguousarray(shard(inputs["w_hh2"]).T), 8, GS).astype(np.float16),
            "wproj": chunked(np.ascontiguousarray(w_proj.T), 8, K).astype(np.float16),
            "wscore": chunked(np.ascontiguousarray(w_score.T), 8, V),
            "embg": embg.astype(np.float16),
            "bias1": (shard(inputs["b_ih1"]) + shard(inputs["b_hh1"]))[None, :].astype(np.float16),
            "bias2": (shard(inputs["b_ih2"]) + shard(inputs["b_hh2"]))[None, :].astype(np.float16),
            "bproj": b_proj[None, :].astype(np.float16),
            "bscore": b_score[:, None].astype(f32),
            "oh": oh.astype(np.float16),
            "sel": sel,
            "eye64": eye64,
            "eye8": eye8,
            "keys_t": kb,
            "vals_t": vb,
        })
    return in_maps


def kernel(**inputs):
    T = int(np.asarray(inputs["labels"]).shape[1])
    in_maps = _prep_inputs(inputs, T)
    if T not in _cache:
        _cache[T] = _build(T)
    nc = _cache[T]
    res = run_bass_kernel_spmd(nc, in_maps, core_ids=list(range(N_CORES)))
    preds = np.empty((B, T, V), np.float32)
    attn = np.empty((B, T, S), np.float32)
    half = T // 2
    for j in range(N_CORES):
        sc = res.results[j]["scores_out"].reshape(V, 2, half, BL)
        ao = res.results[j]["attn_out"]  # [T, BL, S]
        for i in range(BL):
            b = j * BL + i
            preds[b, :half] = sc[:, 0, :, i].T
            preds[b, half:] = sc[:, 1, :, i].T
            attn[b] = ao[:, i, :]
    return preds, attn


# revision 11
# speedup vs baseline: 1.0002x; 1.0002x over previous
"""Trainium2 Bass kernel for nn_Decoder: 3-layer LSTM decoder with attention.

Strategy: tensor-parallel over the hidden/gate dim across 8 cores (each core
holds a 512-row gate slice of every LSTM cell, fp32, resident in SBUF),
with 4 small AllGathers per timestep (h1,h2,h3,ctx — all sent transposed so
the gathered result is directly usable as matmul lhsT chunks). Attention is
batch-sharded (8 batch elements per core, keys/values bf16 in SBUF). The
final score projection is hoisted out of the time loop and computed as one
batched matmul at the end. Softmax uses sigmoid (exp(x) = s/(1-s)) so the
whole kernel uses one ACT table set.
"""
import numpy as np
import concourse.bass as bass
import concourse.bacc as bacc
import concourse.mybir as mybir
import concourse.tile as tile
from concourse.bass_utils import run_bass_kernel_spmd

N_CORES = 8
B, S, E, H, K, V = 64, 400, 512, 1024, 512, 34
BL = B // N_CORES       # 8 local batch elements (attention)
GS = 4 * H // N_CORES   # 512 gate rows per core
HS = H // N_CORES       # 128 h-units per core
F32 = mybir.dt.float32
F16 = mybir.dt.float16
SIG = mybir.ActivationFunctionType.Sigmoid
TANH = mybir.ActivationFunctionType.Tanh
COPY = mybir.ActivationFunctionType.Copy
AX = mybir.AxisListType.X
MUL = mybir.AluOpType.mult
ADD = mybir.AluOpType.add

_cache = {}


def _build(T):
    nc = bacc.Bacc("TRN2", target_bir_lowering=False, debug=False,
                   enable_asserts=False, num_devices=N_CORES)
    dt = nc.dram_tensor
    # per-core weight shards (transposed, chunked on contract dim)
    wctx_d = dt("wctx", [4, 128, GS], F16, kind="ExternalInput")
    whh0_d = dt("whh0", [8, 128, GS], F16, kind="ExternalInput")
    wih1_d = dt("wih1", [8, 128, GS], F16, kind="ExternalInput")
    whh1_d = dt("whh1", [8, 128, GS], F16, kind="ExternalInput")
    wih2_d = dt("wih2", [8, 128, GS], F16, kind="ExternalInput")
    whh2_d = dt("whh2", [8, 128, GS], F16, kind="ExternalInput")
    wproj_d = dt("wproj", [8, 128, K], F16, kind="ExternalInput")
    wscore_d = dt("wscore", [8, 128, V], F32, kind="ExternalInput")
    embg_d = dt("embg", [V + 1, GS], F16, kind="ExternalInput")
    bias1_d = dt("bias1", [1, GS], F16, kind="ExternalInput")
    bias2_d = dt("bias2", [1, GS], F16, kind="ExternalInput")
    bproj_d = dt("bproj", [1, K], F16, kind="ExternalInput")
    bscore_d = dt("bscore", [V, 1], F32, kind="ExternalInput")
    oh_d = dt("oh", [T, V + 1, B], F16, kind="ExternalInput")
    sel_d = dt("sel", [B, BL], F32, kind="ExternalInput")
    eye64_d = dt("eye64", [B, B], F32, kind="ExternalInput")
    eye8_d = dt("eye8", [BL, BL], F32, kind="ExternalInput")
    keys_d = dt("keys_t", [BL, 4, 128, S], F16, kind="ExternalInput")
    vals_d = dt("vals_t", [BL, 4, 128, K], F16, kind="ExternalInput")
    attn_o = dt("attn_out", [T, BL, S], F32, kind="ExternalOutput")
    score_o = dt("scores_out", [V, T * BL], F32, kind="ExternalOutput")

    with tile.TileContext(nc) as tc:
        with (
            tc.tile_pool(name="const", bufs=1) as cp,
            tc.tile_pool(name="work", bufs=2) as wp,
            tc.tile_pool(name="gath", bufs=2) as gp,
            tc.tile_pool(name="ps", bufs=1, space="PSUM") as pp,
            tc.tile_pool(name="dram", bufs=2, space="DRAM") as dp,
        ):
            # ---- resident tensors ----
            wctx = cp.tile([128, 4, GS], F16)
            nc.sync.dma_start(wctx[:], wctx_d.rearrange("c p g -> p c g"))
            whh0 = cp.tile([128, 8, GS], F16)
            nc.sync.dma_start(whh0[:], whh0_d.rearrange("c p g -> p c g"))
            wih1 = cp.tile([128, 8, GS], F16)
            nc.sync.dma_start(wih1[:], wih1_d.rearrange("c p g -> p c g"))
            whh1 = cp.tile([128, 8, GS], F16)
            nc.sync.dma_start(whh1[:], whh1_d.rearrange("c p g -> p c g"))
            wih2 = cp.tile([128, 8, GS], F16)
            nc.sync.dma_start(wih2[:], wih2_d.rearrange("c p g -> p c g"))
            whh2 = cp.tile([128, 8, GS], F16)
            nc.sync.dma_start(whh2[:], whh2_d.rearrange("c p g -> p c g"))
            wproj = cp.tile([128, 8, K], F16)
            nc.sync.dma_start(wproj[:], wproj_d.rearrange("c p g -> p c g"))
            wscore = cp.tile([128, 8, V], F32)
            nc.sync.dma_start(wscore[:], wscore_d.rearrange("c p g -> p c g"))
            embg = cp.tile([V + 1, GS], F16)
            nc.sync.dma_start(embg[:], embg_d[:])
            bias1 = cp.tile([1, GS], F16)
            nc.sync.dma_start(bias1[:], bias1_d[:])
            bias2 = cp.tile([1, GS], F16)
            nc.sync.dma_start(bias2[:], bias2_d[:])
            bproj = cp.tile([1, K], F16)
            nc.sync.dma_start(bproj[:], bproj_d[:])
            bscore = cp.tile([V, 1], F32)
            nc.sync.dma_start(bscore[:], bscore_d[:])
            sel = cp.tile([B, BL], F32)
            nc.sync.dma_start(sel[:], sel_d[:])
            eye64 = cp.tile([B, B], F32)
            nc.sync.dma_start(eye64[:], eye64_d[:])
            eye8 = cp.tile([BL, BL], F32)
            nc.sync.dma_start(eye8[:], eye8_d[:])
            keyst = cp.tile([128, BL, 4, S], F16)
            nc.sync.dma_start(keyst[:], keys_d.rearrange("b c p s -> p b c s"))
            valst = cp.tile([128, BL, 4, K], F16)
            nc.sync.dma_start(valst[:], vals_d.rearrange("b c p s -> p b c s"))
            ones = cp.tile([1, B], F16)
            nc.vector.memset(ones[:], 1.0)
            zrow = cp.tile([1, GS], F16)
            nc.vector.memset(zrow[:], 0.0)
            # LSTM cell state (updated in place each step)
            c1 = cp.tile([B, HS], F32)
            nc.vector.memset(c1[:], 0.0)
            c2 = cp.tile([B, HS], F32)
            nc.vector.memset(c2[:], 0.0)
            c3 = cp.tile([B, HS], F32)
            nc.vector.memset(c3[:], 0.0)
            # DRAM accumulators for deferred score matmul
            qT_acc = dp.tile([T, 128, 4 * BL], F32, bufs=1)
            cT_acc = dp.tile([T, 128, 4 * BL], F32, bufs=1)

            prev = {"h1T": None, "h2T": None, "h3T": None, "ctxT": None}

            def gather_hT(h, tag):
                """transpose h [64,HS] -> [HS,64], allgather -> [128, 8, 64]."""
                tp_ps = pp.tile([128, B], F32, tag="tp", bufs=2)
                nc.tensor.matmul(tp_ps[:], h[:], eye64[:], start=True, stop=True)
                hT = wp.tile([128, B], F16, tag=f"hT{tag}", bufs=2)
                nc.vector.tensor_copy(hT[:], tp_ps[:])
                bin_ = dp.tile([128, B], F16, tag=f"bi{tag}", name=f"bi{tag}")
                nc.sync.dma_start(bin_[:], hT[:])
                bout = dp.tile([N_CORES, 128, B], F16, addr_space="Shared",
                               tag=f"bo{tag}", name=f"bo{tag}")
                nc.gpsimd.collective_compute(
                    "AllGather", mybir.AluOpType.bypass,
                    replica_groups=[list(range(N_CORES))],
                    ins=[bin_.opt()], outs=[bout.opt()])
                hT_all = gp.tile([128, N_CORES, B], F16, tag=f"ga{tag}")
                nc.sync.dma_start(hT_all[:], bout.rearrange("c p b -> p c b"))
                return hT_all

            def cell_nl(g_ps, c_st, tag):
                """gates psum [64, GS] (i|f|o|g) + state -> h [64, HS]."""
                sfo = wp.tile([B, 3 * HS], F32, tag=f"sfo{tag}")
                nc.scalar.activation(sfo[:], g_ps[:, 0:3 * HS], SIG)
                tg = wp.tile([B, HS], F32, tag=f"tg{tag}")
                nc.scalar.activation(tg[:], g_ps[:, 3 * HS:GS], TANH)
                t1 = wp.tile([B, HS], F32, tag=f"t1{tag}")
                nc.vector.tensor_tensor(t1[:], sfo[:, HS:2 * HS], c_st[:], MUL)
                t2 = wp.tile([B, HS], F32, tag=f"t2{tag}")
                nc.vector.tensor_tensor(t2[:], sfo[:, 0:HS], tg[:], MUL)
                nc.vector.tensor_tensor(c_st[:], t1[:], t2[:], ADD)
                tc_ = wp.tile([B, HS], F32, tag=f"tc{tag}")
                nc.scalar.activation(tc_[:], c_st[:], TANH)
                h = wp.tile([B, HS], F32, tag=f"h{tag}", bufs=2)
                nc.vector.tensor_tensor(h[:], sfo[:, 2 * HS:3 * HS], tc_[:], MUL)
                return h

            def pre_emit(t, h1T_all, h2T_all, h3T_all):
                """Start step t's gate psums with everything that doesn't
                need step t's AllGathers (emb one-hot, hh parts, biases).
                Emitted during step t-1's attention so the PE fills AG gaps."""
                oh_t = wp.tile([V + 1, B], F16, tag="oh")
                nc.sync.dma_start(oh_t[:], oh_d[t])
                g0 = pp.tile([B, GS], F32, tag="gates", bufs=3, name=f"g0_{t}")
                nc.tensor.matmul(g0[:], oh_t[:], embg[:], start=True, stop=False)
                if h1T_all is not None:
                    for c in range(8):
                        nc.tensor.matmul(g0[:], h1T_all[:, c, :], whh0[:, c, :],
                                         start=False, stop=False)
                g1 = pp.tile([B, GS], F32, tag="gates", bufs=3, name=f"g1_{t}")
                nc.tensor.matmul(g1[:], ones[:], bias1[:], start=True, stop=False)
                if h2T_all is not None:
                    for c in range(8):
                        nc.tensor.matmul(g1[:], h2T_all[:, c, :], whh1[:, c, :],
                                         start=False, stop=False)
                g2 = pp.tile([B, GS], F32, tag="gates", bufs=3, name=f"g2_{t}")
                nc.tensor.matmul(g2[:], ones[:], bias2[:], start=True, stop=False)
                if h3T_all is not None:
                    for c in range(8):
                        nc.tensor.matmul(g2[:], h3T_all[:, c, :], whh2[:, c, :],
                                         start=False, stop=False)
                return {"g0": g0, "g1": g1, "g2": g2}

            pend = pre_emit(0, None, None, None)
            for t in range(T):
                # ---- cell 0: pre-started psum + ctx(t-1) part ----
                g0 = pend["g0"]
                if t > 0:
                    for c in range(4):
                        nc.tensor.matmul(g0[:], prev["ctxT"][:, c, :],
                                         wctx[:, c, :], start=False,
                                         stop=(c == 3))
                else:
                    nc.tensor.matmul(g0[:], ones[:], zrow[:],
                                     start=False, stop=True)
                h1 = cell_nl(g0, c1, "a")
                h1T_all = gather_hT(h1, "h1")

                # ---- cell 1: pre-started psum + ih part ----
                g1 = pend["g1"]
                for c in range(8):
                    nc.tensor.matmul(g1[:], h1T_all[:, c, :], wih1[:, c, :],
                                     start=False, stop=(c == 7))
                h2 = cell_nl(g1, c2, "b")
                h2T_all = gather_hT(h2, "h2")

                # ---- cell 2 ----
                g2 = pend["g2"]
                for c in range(8):
                    nc.tensor.matmul(g2[:], h2T_all[:, c, :], wih2[:, c, :],
                                     start=False, stop=(c == 7))
                h3 = cell_nl(g2, c3, "c")
                h3T_all = gather_hT(h3, "h3")

                # ---- q = h3 @ w_proj.T + b_proj (replicated) ----
                q_ps = pp.tile([B, K], F32, tag="gates", bufs=3)
                nc.tensor.matmul(q_ps[:], ones[:], bproj[:], start=True, stop=False)
                for c in range(8):
                    nc.tensor.matmul(q_ps[:], h3T_all[:, c, :], wproj[:, c, :],
                                     start=False, stop=(c == 7))
                q_sb = wp.tile([B, K], F32, tag="qsb")
                nc.scalar.activation(q_sb[:], q_ps[:], COPY)
                # select my 8 batches + transpose: qT [128, 4, 8]
                qt_ps = pp.tile([128, 4 * BL], F32, tag="tp", bufs=2)
                for c in range(4):
                    nc.tensor.matmul(qt_ps[:, c * BL:(c + 1) * BL],
                                     q_sb[:, c * 128:(c + 1) * 128], sel[:],
                                     start=True, stop=True)
                qT_bf = wp.tile([128, 4 * BL], F16, tag="qTbf")
                nc.vector.tensor_copy(qT_bf[:], qt_ps[:])
                qT_f = wp.tile([128, 4 * BL], F32, tag="qTf")
                nc.vector.tensor_copy(qT_f[:], qt_ps[:])
                nc.sync.dma_start(qT_acc[t], qT_f[:])

                # ---- attention energies (per-b matvec, col-tiled 4x) ----
                e_ps = [pp.tile([128, S], F32, tag="att", bufs=3, name=f"e{g}_{t}")
                        for g in range(2)]
                for b in range(BL):
                    ps = e_ps[b // 4]
                    row = 32 * (b % 4)
                    for c in range(4):
                        nc.tensor.matmul(
                            ps[row:row + 1, :],
                            qT_bf[:, c * BL + b:c * BL + b + 1],
                            keyst[:, b, c, :],
                            start=(c == 0), stop=(c == 3),
                            tile_position=(0, row))
                if t + 1 < T:
                    pend = pre_emit(t + 1, h1T_all, h2T_all, h3T_all)

                # ---- compact energies into [8, S], then sigmoid-softmax ----
                e_sb = wp.tile([128, 2, S], F32, tag="esb", bufs=1)
                for b in range(BL):
                    g, row = b // 4, 32 * (b % 4)
                    nc.vector.tensor_copy(e_sb[row:row + 1, g, :],
                                          e_ps[g][row:row + 1, :])
                e8 = wp.tile([BL, S], F32, tag="e8")
                for g in range(2):
                    ev = e_sb.rearrange("(a b) g s -> a b g s", b=32)[:, 0, g, :]
                    nc.sync.dma_start(e8[g * 4:(g + 1) * 4, :], ev)
                m8 = wp.tile([BL, 1], F32, tag="m8")
                nc.vector.reduce_max(m8[:], e8[:], axis=AX)
                nm8 = wp.tile([BL, 1], F32, tag="nm8")
                nc.vector.tensor_scalar_mul(nm8[:], m8[:], -1.0)
                sg8 = wp.tile([BL, S], F32, tag="sg8")
                nc.scalar.activation(sg8[:], e8[:], SIG, bias=nm8[:])
                u8 = wp.tile([BL, S], F32, tag="u8")
                nc.vector.tensor_scalar(u8[:], sg8[:], -1.0, 1.0, op0=MUL, op1=ADD)
                nc.vector.reciprocal(u8[:], u8[:])
                attn8 = wp.tile([BL, S], F32, tag="attn8")
                nc.vector.tensor_tensor(attn8[:], sg8[:], u8[:], MUL)  # exp(y)
                z8 = wp.tile([BL, 1], F32, tag="z8")
                nc.vector.reduce_sum(z8[:], attn8[:], axis=AX)
                nc.vector.reciprocal(z8[:], z8[:])
                nc.vector.tensor_scalar_mul(attn8[:], attn8[:], z8[:])
                nc.sync.dma_start(attn_o[t], attn8[:])

                # ---- transpose attn -> attnT [128, 4, 8] (bf16) ----
                at_ps = pp.tile([128, 4 * BL], F32, tag="tp", bufs=2)
                for c in range(4):
                    pr = 128 if c < 3 else S - 3 * 128
                    nc.tensor.matmul(at_ps[0:pr, c * BL:(c + 1) * BL],
                                     attn8[:, c * 128:c * 128 + pr], eye8[:],
                                     start=True, stop=True)
                aT_bf = wp.tile([128, 4 * BL], F16, tag="aTbf")
                nc.vector.memset(aT_bf[:], 0.0)
                nc.vector.tensor_copy(aT_bf[:, 0:3 * BL], at_ps[:, 0:3 * BL])
                nc.vector.tensor_copy(aT_bf[0:S - 3 * 128, 3 * BL:4 * BL],
                                      at_ps[0:S - 3 * 128, 3 * BL:4 * BL])

                # ---- ctx = attn @ V (per-b, col-tiled) ----
                c_ps = [pp.tile([128, K], F32, tag="att", bufs=3, name=f"c{g}_{t}")
                        for g in range(2)]
                for b in range(BL):
                    ps = c_ps[b // 4]
                    row = 32 * (b % 4)
                    for c in range(4):
                        nc.tensor.matmul(
                            ps[row:row + 1, :],
                            aT_bf[:, c * BL + b:c * BL + b + 1],
                            valst[:, b, c, :],
                            start=(c == 0), stop=(c == 3),
                            tile_position=(0, row))
                # compact ctx rows into [8, K]
                c_sb = wp.tile([128, 2, K], F32, tag="csb", bufs=1)
                for b in range(BL):
                    g, row = b // 4, 32 * (b % 4)
                    nc.vector.tensor_copy(c_sb[row:row + 1, g, :],
                                          c_ps[g][row:row + 1, :])
                ctx8 = wp.tile([BL, K], F32, tag="ctx8")
                for g in range(2):
                    cv = c_sb.rearrange("(a b) g s -> a b g s", b=32)[:, 0, g, :]
                    nc.sync.dma_start(ctx8[g * 4:(g + 1) * 4, :], cv)
                # transpose ctx -> ctxT [128, 4, 8]
                ct_ps = pp.tile([128, 4 * BL], F32, tag="tp", bufs=2)
                for c in range(4):
                    nc.tensor.matmul(ct_ps[:, c * BL:(c + 1) * BL],
                                     ctx8[:, c * 128:(c + 1) * 128], eye8[:],
                                     start=True, stop=True)
                cT_f = wp.tile([128, 4 * BL], F32, tag="cTf")
                nc.vector.tensor_copy(cT_f[:], ct_ps[:])
                nc.sync.dma_start(cT_acc[t], cT_f[:])
                cT_h = wp.tile([128, 4 * BL], F16, tag="cTh")
                nc.vector.tensor_copy(cT_h[:], ct_ps[:])
                # allgather ctxT
                cbin = dp.tile([128, 4 * BL], F16, tag="cbi", name=f"cbi_{t}")
                nc.sync.dma_start(cbin[:], cT_h[:])
                cbout = dp.tile([N_CORES, 128, 4 * BL], F16, addr_space="Shared",
                                tag="cbo", name=f"cbo_{t}")
                nc.gpsimd.collective_compute(
                    "AllGather", mybir.AluOpType.bypass,
                    replica_groups=[list(range(N_CORES))],
                    ins=[cbin.opt()], outs=[cbout.opt()])
                ctxT_all = gp.tile([128, 4, N_CORES, BL], F16, tag="gactx")
                nc.sync.dma_start(
                    ctxT_all[:],
                    cbout.rearrange("c p (k b) -> p k c b", b=BL))

                prev = {"h1T": h1T_all, "h2T": h2T_all, "h3T": h3T_all,
                        "ctxT": ctxT_all.rearrange("p k c b -> p k (c b)")}

            # ---- deferred scores: [V, T*BL] = wscore.T @ [q;ctx] + b ----
            sc_sb = wp.tile([V, T * BL], F32, tag="scsb", bufs=1)
            for hf in range(2):
                t0, t1_ = hf * (T // 2), (hf + 1) * (T // 2)
                n = (t1_ - t0) * BL
                s_ps = pp.tile([V, n], F32, tag="att", bufs=3, name=f"sps{hf}")
                pairs = [(si, c) for si in range(2) for c in range(4)]
                for idx, (si, c) in enumerate(pairs):
                    src = (qT_acc, cT_acc)[si]
                    rhs_t = wp.tile([128, t1_ - t0, BL], F32, tag="srhs")
                    nc.sync.dma_start(
                        rhs_t[:],
                        src[t0:t1_].rearrange("t p (k b) -> p k t b", b=BL)
                        [:, c, :, :])
                    nc.tensor.matmul(s_ps[:], wscore[:, c + 4 * si, :],
                                     rhs_t[:], start=(idx == 0),
                                     stop=(idx == len(pairs) - 1))
                nc.scalar.activation(sc_sb[:, t0 * BL:t1_ * BL], s_ps[:],
                                     mybir.ActivationFunctionType.Identity,
                                     bias=bscore[:])
            nc.sync.dma_start(score_o[:], sc_sb[:])

    nc.compile()
    return nc


def _prep_inputs(inputs, T):
    """Build per-core in_maps from full inputs."""
    f32 = np.float32
    labels = np.asarray(inputs["labels"]).astype(np.int64)
    keys = np.asarray(inputs["keys"], f32)
    values = np.asarray(inputs["values"], f32)
    emb = np.asarray(inputs["emb"], f32)
    w_proj = np.asarray(inputs["w_proj"], f32)
    b_proj = np.asarray(inputs["b_proj"], f32)
    w_score = np.asarray(inputs["w_score"], f32)
    b_score = np.asarray(inputs["b_score"], f32)
    h0 = np.asarray(inputs["h0"], f32)

    eye64 = np.eye(B, dtype=f32)
    eye8 = np.eye(BL, dtype=f32)
    oh = np.zeros((T, V + 1, B), f32)
    for t in range(T):
        oh[t, labels[:, t], np.arange(B)] = 1.0
    oh[0, V, :] = 1.0  # initial-context constant row

    # torch gate order i,f,g,o ; our section order i,f,o,g
    blk = {"i": 0, "f": 1, "g": 2, "o": 3}
    in_maps = []
    ctx0 = (h0[0] @ w_proj.T + b_proj).astype(f32)  # [K], h0 rows identical
    for j in range(N_CORES):
        rows = np.concatenate([
            np.arange(blk[g] * H + j * HS, blk[g] * H + (j + 1) * HS)
            for g in ("i", "f", "o", "g")])

        def shard(w):
            return np.ascontiguousarray(np.asarray(w, f32)[rows])

        wih0_s = shard(inputs["w_ih0"])      # [GS, E+K]
        whh0_s = shard(inputs["w_hh0"])      # [GS, H]
        b0 = shard(inputs["b_ih0"]) + shard(inputs["b_hh0"])
        embg = np.concatenate([
            emb @ wih0_s[:, :E].T + b0[None, :],
            (ctx0 @ wih0_s[:, E:].T + 0.0)[None, :]], axis=0).astype(f32)

        def chunked(wT, nch, width):
            return np.ascontiguousarray(wT.reshape(nch, 128, width))

        sel = np.zeros((B, BL), f32)
        sel[np.arange(j * BL, (j + 1) * BL), np.arange(BL)] = 1.0

        kb = np.zeros((BL, 4, 128, S), np.float16)
        vb = np.zeros((BL, 4, 128, K), np.float16)
        for i in range(BL):
            b = j * BL + i
            kb[i] = keys[:, b, :].T.reshape(4, 128, S).astype(np.float16)
            vpad = np.zeros((512, K), f32)
            vpad[:S] = values[:, b, :]
            vb[i] = vpad.reshape(4, 128, K).astype(np.float16)

        in_maps.append({
            "wctx": chunked(np.ascontiguousarray(wih0_s[:, E:].T), 4, GS).astype(np.float16),
            "whh0": chunked(np.ascontiguousarray(whh0_s.T), 8, GS).astype(np.float16),
            "wih1": chunked(np.ascontiguousarray(shard(inputs["w_ih1"]).T), 8, GS).astype(np.float16),
            "whh1": chunked(np.ascontiguousarray(shard(inputs["w_hh1"]).T), 8, GS).astype(np.float16),
            "wih2": chunked(np.ascontiguousarray(shard(inputs["w_ih2"]).T), 8, GS).astype(np.float16),
            "whh2": chunked(np.ascontiguousarray(shard(inputs["w_hh2"]).T), 8, GS).astype(np.float16),
            "wproj": chunked(np.ascontiguousarray(w_proj.T), 8, K).astype(np.float16),
            "wscore": chunked(np.ascontiguousarray(w_score.T), 8, V),
            "embg": embg.astype(np.float16),
            "bias1": (shard(inputs["b_ih1"]) + shard(inputs["b_hh1"]))[None, :].astype(np.float16),
            "bias2": (shard(inputs["b_ih2"]) + shard(inputs["b_hh2"]))[None, :].astype(np.float16),
            "bproj": b_proj[None, :].astype(np.float16),
            "bscore": b_score[:, None].astype(f32),
            "oh": oh.astype(np.float16),
            "sel": sel,
            "eye64": eye64,
            "eye8": eye8,
            "keys_t": kb,
            "vals_t": vb,
        })
    return in_maps


def kernel(**inputs):
    T = int(np.asarray(inputs["labels"]).shape[1])
    in_maps = _prep_inputs(inputs, T)
    if T not in _cache:
        _cache[T] = _build(T)
    nc = _cache[T]
    res = run_bass_kernel_spmd(nc, in_maps, core_ids=list(range(N_CORES)))
    preds = np.empty((B, T, V), np.float32)
    attn = np.empty((B, T, S), np.float32)
    half = T // 2
    for j in range(N_CORES):
        sc = res.results[j]["scores_out"].reshape(V, 2, half, BL)
        ao = res.results[j]["attn_out"]  # [T, BL, S]
        for i in range(BL):
            b = j * BL + i
            preds[b, :half] = sc[:, 0, :, i].T
            preds[b, half:] = sc[:, 1, :, i].T
            attn[b] = ao[:, i, :]
    return preds, attn


# revision 12
# speedup vs baseline: 1.0096x; 1.0095x over previous
"""Trainium2 Bass kernel for nn_Decoder: 3-layer LSTM decoder with attention.

Strategy: tensor-parallel over the hidden/gate dim across 8 cores (each core
holds a 512-row gate slice of every LSTM cell, fp32, resident in SBUF),
with 4 small AllGathers per timestep (h1,h2,h3,ctx — all sent transposed so
the gathered result is directly usable as matmul lhsT chunks). Attention is
batch-sharded (8 batch elements per core, keys/values bf16 in SBUF). The
final score projection is hoisted out of the time loop and computed as one
batched matmul at the end. Softmax uses sigmoid (exp(x) = s/(1-s)) so the
whole kernel uses one ACT table set.
"""
import numpy as np
import concourse.bass as bass
import concourse.bacc as bacc
import concourse.mybir as mybir
import concourse.tile as tile
from concourse.bass_utils import run_bass_kernel_spmd

N_CORES = 8
B, S, E, H, K, V = 64, 400, 512, 1024, 512, 34
BL = B // N_CORES       # 8 local batch elements (attention)
GS = 4 * H // N_CORES   # 512 gate rows per core
HS = H // N_CORES       # 128 h-units per core
F32 = mybir.dt.float32
F16 = mybir.dt.float16
SIG = mybir.ActivationFunctionType.Sigmoid
TANH = mybir.ActivationFunctionType.Tanh
COPY = mybir.ActivationFunctionType.Copy
AX = mybir.AxisListType.X
MUL = mybir.AluOpType.mult
ADD = mybir.AluOpType.add

_cache = {}


def _build(T):
    nc = bacc.Bacc("TRN2", target_bir_lowering=False, debug=False,
                   enable_asserts=False, num_devices=N_CORES)
    dt = nc.dram_tensor
    # per-core weight shards (transposed, chunked on contract dim)
    wctx_d = dt("wctx", [4, 128, GS], F16, kind="ExternalInput")
    whh0_d = dt("whh0", [8, 128, GS], F16, kind="ExternalInput")
    wih1_d = dt("wih1", [8, 128, GS], F16, kind="ExternalInput")
    whh1_d = dt("whh1", [8, 128, GS], F16, kind="ExternalInput")
    wih2_d = dt("wih2", [8, 128, GS], F16, kind="ExternalInput")
    whh2_d = dt("whh2", [8, 128, GS], F16, kind="ExternalInput")
    wproj_d = dt("wproj", [8, 128, K], F16, kind="ExternalInput")
    wscore_d = dt("wscore", [8, 128, V], F32, kind="ExternalInput")
    embg_d = dt("embg", [V + 1, GS], F16, kind="ExternalInput")
    bias1_d = dt("bias1", [1, GS], F16, kind="ExternalInput")
    bias2_d = dt("bias2", [1, GS], F16, kind="ExternalInput")
    bproj_d = dt("bproj", [1, K], F16, kind="ExternalInput")
    bscore_d = dt("bscore", [V, 1], F32, kind="ExternalInput")
    oh_d = dt("oh", [T, V + 1, B], F16, kind="ExternalInput")
    sel_d = dt("sel", [B, BL], F32, kind="ExternalInput")
    eye64_d = dt("eye64", [B, B], F32, kind="ExternalInput")
    eye8_d = dt("eye8", [BL, BL], F32, kind="ExternalInput")
    keys_d = dt("keys_t", [BL, 4, 128, S], F16, kind="ExternalInput")
    vals_d = dt("vals_t", [BL, 4, 128, K], F16, kind="ExternalInput")
    attn_o = dt("attn_out", [T, BL, S], F32, kind="ExternalOutput")
    score_o = dt("scores_out", [V, T * BL], F32, kind="ExternalOutput")

    with tile.TileContext(nc) as tc:
        with (
            tc.tile_pool(name="const", bufs=1) as cp,
            tc.tile_pool(name="work", bufs=2) as wp,
            tc.tile_pool(name="gath", bufs=2) as gp,
            tc.tile_pool(name="ps", bufs=1, space="PSUM") as pp,
            tc.tile_pool(name="dram", bufs=2, space="DRAM") as dp,
        ):
            # ---- resident tensors ----
            wctx = cp.tile([128, 4, GS], F16)
            nc.sync.dma_start(wctx[:], wctx_d.rearrange("c p g -> p c g"))
            whh0 = cp.tile([128, 8, GS], F16)
            nc.sync.dma_start(whh0[:], whh0_d.rearrange("c p g -> p c g"))
            wih1 = cp.tile([128, 8, GS], F16)
            nc.sync.dma_start(wih1[:], wih1_d.rearrange("c p g -> p c g"))
            whh1 = cp.tile([128, 8, GS], F16)
            nc.sync.dma_start(whh1[:], whh1_d.rearrange("c p g -> p c g"))
            wih2 = cp.tile([128, 8, GS], F16)
            nc.sync.dma_start(wih2[:], wih2_d.rearrange("c p g -> p c g"))
            whh2 = cp.tile([128, 8, GS], F16)
            nc.sync.dma_start(whh2[:], whh2_d.rearrange("c p g -> p c g"))
            wproj = cp.tile([128, 8, K], F16)
            nc.sync.dma_start(wproj[:], wproj_d.rearrange("c p g -> p c g"))
            wscore = cp.tile([128, 8, V], F32)
            nc.sync.dma_start(wscore[:], wscore_d.rearrange("c p g -> p c g"))
            embg = cp.tile([V + 1, GS], F16)
            nc.sync.dma_start(embg[:], embg_d[:])
            bias1 = cp.tile([1, GS], F16)
            nc.sync.dma_start(bias1[:], bias1_d[:])
            bias2 = cp.tile([1, GS], F16)
            nc.sync.dma_start(bias2[:], bias2_d[:])
            bproj = cp.tile([1, K], F16)
            nc.sync.dma_start(bproj[:], bproj_d[:])
            bscore = cp.tile([V, 1], F32)
            nc.sync.dma_start(bscore[:], bscore_d[:])
            sel = cp.tile([B, BL], F32)
            nc.sync.dma_start(sel[:], sel_d[:])
            eye64 = cp.tile([B, B], F32)
            nc.sync.dma_start(eye64[:], eye64_d[:])
            eye8 = cp.tile([BL, BL], F32)
            nc.sync.dma_start(eye8[:], eye8_d[:])
            keyst = cp.tile([128, BL, 4, S], F16)
            nc.sync.dma_start(keyst[:], keys_d.rearrange("b c p s -> p b c s"))
            valst = cp.tile([128, BL, 4, K], F16)
            nc.sync.dma_start(valst[:], vals_d.rearrange("b c p s -> p b c s"))
            ones = cp.tile([1, B], F16)
            nc.vector.memset(ones[:], 1.0)
            zrow = cp.tile([1, GS], F16)
            nc.vector.memset(zrow[:], 0.0)
            # LSTM cell state (updated in place each step)
            c1 = cp.tile([B, HS], F32)
            nc.vector.memset(c1[:], 0.0)
            c2 = cp.tile([B, HS], F32)
            nc.vector.memset(c2[:], 0.0)
            c3 = cp.tile([B, HS], F32)
            nc.vector.memset(c3[:], 0.0)
            # DRAM accumulators for deferred score matmul
            qT_acc = dp.tile([T, 128, 4 * BL], F32, bufs=1)
            cT_acc = dp.tile([T, 128, 4 * BL], F32, bufs=1)

            prev = {"h1T": None, "h2T": None, "h3T": None, "ctxT": None}

            def gather_hT(h, tag):
                """transpose h [64,HS] -> [HS,64], allgather -> [128, 8, 64]."""
                tp_ps = pp.tile([128, B], F32, tag="tp", bufs=2)
                nc.tensor.matmul(tp_ps[:], h[:], eye64[:], start=True, stop=True)
                hT = wp.tile([128, B], F16, tag=f"hT{tag}", bufs=2)
                nc.vector.tensor_copy(hT[:], tp_ps[:])
                bin_ = dp.tile([128, B], F16, tag=f"bi{tag}", name=f"bi{tag}")
                nc.sync.dma_start(bin_[:], hT[:])
                bout = dp.tile([N_CORES, 128, B], F16, addr_space="Shared",
                               tag=f"bo{tag}", name=f"bo{tag}")
                nc.gpsimd.collective_compute(
                    "AllGather", mybir.AluOpType.bypass,
                    replica_groups=[list(range(N_CORES))],
                    ins=[bin_.opt()], outs=[bout.opt()])
                hT_all = gp.tile([128, N_CORES, B], F16, tag=f"ga{tag}")
                nc.sync.dma_start(hT_all[:], bout.rearrange("c p b -> p c b"))
                return hT_all

            def cell_nl(g_ps, c_st, tag):
                """gates psum [64, GS] (i|f|o|g) + state -> h [64, HS]."""
                sfo = wp.tile([B, 3 * HS], F32, tag=f"sfo{tag}")
                nc.scalar.activation(sfo[:], g_ps[:, 0:3 * HS], SIG)
                tg = wp.tile([B, HS], F32, tag=f"tg{tag}")
                nc.scalar.activation(tg[:], g_ps[:, 3 * HS:GS], TANH)
                t1 = wp.tile([B, HS], F32, tag=f"t1{tag}")
                nc.vector.tensor_tensor(t1[:], sfo[:, HS:2 * HS], c_st[:], MUL)
                t2 = wp.tile([B, HS], F32, tag=f"t2{tag}")
                nc.vector.tensor_tensor(t2[:], sfo[:, 0:HS], tg[:], MUL)
                nc.vector.tensor_tensor(c_st[:], t1[:], t2[:], ADD)
                tc_ = wp.tile([B, HS], F32, tag=f"tc{tag}")
                nc.scalar.activation(tc_[:], c_st[:], TANH)
                h = wp.tile([B, HS], F32, tag=f"h{tag}", bufs=2)
                nc.vector.tensor_tensor(h[:], sfo[:, 2 * HS:3 * HS], tc_[:], MUL)
                return h

            def start_g0(t):
                """emb one-hot (+ hh0 when h1[t-1] is gathered) into a fresh
                gates psum — emitted inside the previous AG window so the PE
                stays busy (HAM-warm) during the collective."""
                oh_t = wp.tile([V + 1, B], F16, tag="oh")
                nc.sync.dma_start(oh_t[:], oh_d[t])
                g0 = pp.tile([B, GS], F32, tag="gates", bufs=3, name=f"g0_{t}")
                nc.tensor.matmul(g0[:], oh_t[:], embg[:], start=True, stop=False)
                if prev["h1T"] is not None:
                    for c in range(8):
                        nc.tensor.matmul(g0[:], prev["h1T"][:, c, :],
                                         whh0[:, c, :], start=False, stop=False)
                return g0

            g0 = None
            for t in range(T):
                if g0 is None:
                    g0 = start_g0(0)
                # ---- cell 0: pre-started psum + ctx(t-1) part ----
                if t > 0:
                    for c in range(4):
                        nc.tensor.matmul(g0[:], prev["ctxT"][:, c, :],
                                         wctx[:, c, :], start=False,
                                         stop=(c == 3))
                else:
                    nc.tensor.matmul(g0[:], ones[:], zrow[:],
                                     start=False, stop=True)
                h1 = cell_nl(g0, c1, "a")
                h1T_all = gather_hT(h1, "h1")

                # fill AG(h1) window: bias + hh part of cell 1 (deps ready)
                g1 = pp.tile([B, GS], F32, tag="gates", bufs=3, name=f"g1_{t}")
                nc.tensor.matmul(g1[:], ones[:], bias1[:], start=True, stop=False)
                if t > 0:
                    for c in range(8):
                        nc.tensor.matmul(g1[:], prev["h2T"][:, c, :],
                                         whh1[:, c, :], start=False, stop=False)
                for c in range(8):
                    nc.tensor.matmul(g1[:], h1T_all[:, c, :], wih1[:, c, :],
                                     start=False, stop=(c == 7))
                h2 = cell_nl(g1, c2, "b")
                h2T_all = gather_hT(h2, "h2")

                # fill AG(h2) window: bias + hh part of cell 2
                g2 = pp.tile([B, GS], F32, tag="gates", bufs=3, name=f"g2_{t}")
                nc.tensor.matmul(g2[:], ones[:], bias2[:], start=True, stop=False)
                if t > 0:
                    for c in range(8):
                        nc.tensor.matmul(g2[:], prev["h3T"][:, c, :],
                                         whh2[:, c, :], start=False, stop=False)
                for c in range(8):
                    nc.tensor.matmul(g2[:], h2T_all[:, c, :], wih2[:, c, :],
                                     start=False, stop=(c == 7))
                h3 = cell_nl(g2, c3, "c")
                h3T_all = gather_hT(h3, "h3")

                # fill AG(h3) window: next step's emb + hh0 (h1T just gathered)
                prev = {"h1T": h1T_all, "h2T": h2T_all, "h3T": h3T_all,
                        "ctxT": prev["ctxT"]}
                g0 = start_g0(t + 1) if t + 1 < T else None

                # ---- q = h3 @ w_proj.T + b_proj (replicated) ----
                q_ps = pp.tile([B, K], F32, tag="gates", bufs=3)
                nc.tensor.matmul(q_ps[:], ones[:], bproj[:], start=True, stop=False)
                for c in range(8):
                    nc.tensor.matmul(q_ps[:], h3T_all[:, c, :], wproj[:, c, :],
                                     start=False, stop=(c == 7))
                q_sb = wp.tile([B, K], F32, tag="qsb")
                nc.scalar.activation(q_sb[:], q_ps[:], COPY)
                # select my 8 batches + transpose: qT [128, 4, 8]
                qt_ps = pp.tile([128, 4 * BL], F32, tag="tp", bufs=2)
                for c in range(4):
                    nc.tensor.matmul(qt_ps[:, c * BL:(c + 1) * BL],
                                     q_sb[:, c * 128:(c + 1) * 128], sel[:],
                                     start=True, stop=True)
                qT_bf = wp.tile([128, 4 * BL], F16, tag="qTbf")
                nc.vector.tensor_copy(qT_bf[:], qt_ps[:])
                qT_f = wp.tile([128, 4 * BL], F32, tag="qTf")
                nc.vector.tensor_copy(qT_f[:], qt_ps[:])
                nc.sync.dma_start(qT_acc[t], qT_f[:])

                # ---- attention energies (per-b matvec, col-tiled 4x) ----
                e_ps = [pp.tile([128, S], F32, tag="att", bufs=3, name=f"e{g}_{t}")
                        for g in range(2)]
                for b in range(BL):
                    ps = e_ps[b // 4]
                    row = 32 * (b % 4)
                    for c in range(4):
                        nc.tensor.matmul(
                            ps[row:row + 1, :],
                            qT_bf[:, c * BL + b:c * BL + b + 1],
                            keyst[:, b, c, :],
                            start=(c == 0), stop=(c == 3),
                            tile_position=(0, row))
                # ---- compact energies into [8, S], then sigmoid-softmax ----
                e_sb = wp.tile([128, 2, S], F32, tag="esb", bufs=1)
                for b in range(BL):
                    g, row = b // 4, 32 * (b % 4)
                    nc.vector.tensor_copy(e_sb[row:row + 1, g, :],
                                          e_ps[g][row:row + 1, :])
                e8 = wp.tile([BL, S], F32, tag="e8")
                for g in range(2):
                    ev = e_sb.rearrange("(a b) g s -> a b g s", b=32)[:, 0, g, :]
                    nc.sync.dma_start(e8[g * 4:(g + 1) * 4, :], ev)
                m8 = wp.tile([BL, 1], F32, tag="m8")
                nc.vector.reduce_max(m8[:], e8[:], axis=AX)
                nm8 = wp.tile([BL, 1], F32, tag="nm8")
                nc.vector.tensor_scalar_mul(nm8[:], m8[:], -1.0)
                sg8 = wp.tile([BL, S], F32, tag="sg8")
                nc.scalar.activation(sg8[:], e8[:], SIG, bias=nm8[:])
                u8 = wp.tile([BL, S], F32, tag="u8")
                nc.vector.tensor_scalar(u8[:], sg8[:], -1.0, 1.0, op0=MUL, op1=ADD)
                nc.vector.reciprocal(u8[:], u8[:])
                attn8 = wp.tile([BL, S], F32, tag="attn8")
                nc.vector.tensor_tensor(attn8[:], sg8[:], u8[:], MUL)  # exp(y)
                z8 = wp.tile([BL, 1], F32, tag="z8")
                nc.vector.reduce_sum(z8[:], attn8[:], axis=AX)
                nc.vector.reciprocal(z8[:], z8[:])
                nc.vector.tensor_scalar_mul(attn8[:], attn8[:], z8[:])
                nc.sync.dma_start(attn_o[t], attn8[:])

                # ---- transpose attn -> attnT [128, 4, 8] (bf16) ----
                at_ps = pp.tile([128, 4 * BL], F32, tag="tp", bufs=2)
                for c in range(4):
                    pr = 128 if c < 3 else S - 3 * 128
                    nc.tensor.matmul(at_ps[0:pr, c * BL:(c + 1) * BL],
                                     attn8[:, c * 128:c * 128 + pr], eye8[:],
                                     start=True, stop=True)
                aT_bf = wp.tile([128, 4 * BL], F16, tag="aTbf")
                nc.vector.memset(aT_bf[:], 0.0)
                nc.vector.tensor_copy(aT_bf[:, 0:3 * BL], at_ps[:, 0:3 * BL])
                nc.vector.tensor_copy(aT_bf[0:S - 3 * 128, 3 * BL:4 * BL],
                                      at_ps[0:S - 3 * 128, 3 * BL:4 * BL])

                # ---- ctx = attn @ V (per-b, col-tiled) ----
                c_ps = [pp.tile([128, K], F32, tag="att", bufs=3, name=f"c{g}_{t}")
                        for g in range(2)]
                for b in range(BL):
                    ps = c_ps[b // 4]
                    row = 32 * (b % 4)
                    for c in range(4):
                        nc.tensor.matmul(
                            ps[row:row + 1, :],
                            aT_bf[:, c * BL + b:c * BL + b + 1],
                            valst[:, b, c, :],
                            start=(c == 0), stop=(c == 3),
                            tile_position=(0, row))
                # compact ctx rows into [8, K]
                c_sb = wp.tile([128, 2, K], F32, tag="csb", bufs=1)
                for b in range(BL):
                    g, row = b // 4, 32 * (b % 4)
                    nc.vector.tensor_copy(c_sb[row:row + 1, g, :],
                                          c_ps[g][row:row + 1, :])
                ctx8 = wp.tile([BL, K], F32, tag="ctx8")
                for g in range(2):
                    cv = c_sb.rearrange("(a b) g s -> a b g s", b=32)[:, 0, g, :]
                    nc.sync.dma_start(ctx8[g * 4:(g + 1) * 4, :], cv)
                # transpose ctx -> ctxT [128, 4, 8]
                ct_ps = pp.tile([128, 4 * BL], F32, tag="tp", bufs=2)
                for c in range(4):
                    nc.tensor.matmul(ct_ps[:, c * BL:(c + 1) * BL],
                                     ctx8[:, c * 128:(c + 1) * 128], eye8[:],
                                     start=True, stop=True)
                cT_f = wp.tile([128, 4 * BL], F32, tag="cTf")
                nc.vector.tensor_copy(cT_f[:], ct_ps[:])
                nc.sync.dma_start(cT_acc[t], cT_f[:])
                cT_h = wp.tile([128, 4 * BL], F16, tag="cTh")
                nc.vector.tensor_copy(cT_h[:], ct_ps[:])
                # allgather ctxT
                cbin = dp.tile([128, 4 * BL], F16, tag="cbi", name=f"cbi_{t}")
                nc.sync.dma_start(cbin[:], cT_h[:])
                cbout = dp.tile([N_CORES, 128, 4 * BL], F16, addr_space="Shared",
                                tag="cbo", name=f"cbo_{t}")
                nc.gpsimd.collective_compute(
                    "AllGather", mybir.AluOpType.bypass,
                    replica_groups=[list(range(N_CORES))],
                    ins=[cbin.opt()], outs=[cbout.opt()])
                ctxT_all = gp.tile([128, 4, N_CORES, BL], F16, tag="gactx")
                nc.sync.dma_start(
                    ctxT_all[:],
                    cbout.rearrange("c p (k b) -> p k c b", b=BL))

                prev["ctxT"] = ctxT_all.rearrange("p k c b -> p k (c b)")

            # ---- deferred scores: [V, T*BL] = wscore.T @ [q;ctx] + b ----
            sc_sb = wp.tile([V, T * BL], F32, tag="scsb", bufs=1)
            for hf in range(2):
                t0, t1_ = hf * (T // 2), (hf + 1) * (T // 2)
                n = (t1_ - t0) * BL
                s_ps = pp.tile([V, n], F32, tag="att", bufs=3, name=f"sps{hf}")
                pairs = [(si, c) for si in range(2) for c in range(4)]
                for idx, (si, c) in enumerate(pairs):
                    src = (qT_acc, cT_acc)[si]
                    rhs_t = wp.tile([128, t1_ - t0, BL], F32, tag="srhs")
                    nc.sync.dma_start(
                        rhs_t[:],
                        src[t0:t1_].rearrange("t p (k b) -> p k t b", b=BL)
                        [:, c, :, :])
                    nc.tensor.matmul(s_ps[:], wscore[:, c + 4 * si, :],
                                     rhs_t[:], start=(idx == 0),
                                     stop=(idx == len(pairs) - 1))
                nc.scalar.activation(sc_sb[:, t0 * BL:t1_ * BL], s_ps[:],
                                     mybir.ActivationFunctionType.Identity,
                                     bias=bscore[:])
            nc.sync.dma_start(score_o[:], sc_sb[:])

    nc.compile()
    return nc


def _prep_inputs(inputs, T):
    """Build per-core in_maps from full inputs."""
    f32 = np.float32
    labels = np.asarray(inputs["labels"]).astype(np.int64)
    keys = np.asarray(inputs["keys"], f32)
    values = np.asarray(inputs["values"], f32)
    emb = np.asarray(inputs["emb"], f32)
    w_proj = np.asarray(inputs["w_proj"], f32)
    b_proj = np.asarray(inputs["b_proj"], f32)
    w_score = np.asarray(inputs["w_score"], f32)
    b_score = np.asarray(inputs["b_score"], f32)
    h0 = np.asarray(inputs["h0"], f32)

    eye64 = np.eye(B, dtype=f32)
    eye8 = np.eye(BL, dtype=f32)
    oh = np.zeros((T, V + 1, B), f32)
    for t in range(T):
        oh[t, labels[:, t], np.arange(B)] = 1.0
    oh[0, V, :] = 1.0  # initial-context constant row

    # torch gate order i,f,g,o ; our section order i,f,o,g
    blk = {"i": 0, "f": 1, "g": 2, "o": 3}
    in_maps = []
    ctx0 = (h0[0] @ w_proj.T + b_proj).astype(f32)  # [K], h0 rows identical
    for j in range(N_CORES):
        rows = np.concatenate([
            np.arange(blk[g] * H + j * HS, blk[g] * H + (j + 1) * HS)
            for g in ("i", "f", "o", "g")])

        def shard(w):
            return np.ascontiguousarray(np.asarray(w, f32)[rows])

        wih0_s = shard(inputs["w_ih0"])      # [GS, E+K]
        whh0_s = shard(inputs["w_hh0"])      # [GS, H]
        b0 = shard(inputs["b_ih0"]) + shard(inputs["b_hh0"])
        embg = np.concatenate([
            emb @ wih0_s[:, :E].T + b0[None, :],
            (ctx0 @ wih0_s[:, E:].T + 0.0)[None, :]], axis=0).astype(f32)

        def chunked(wT, nch, width):
            return np.ascontiguousarray(wT.reshape(nch, 128, width))

        sel = np.zeros((B, BL), f32)
        sel[np.arange(j * BL, (j + 1) * BL), np.arange(BL)] = 1.0

        kb = np.zeros((BL, 4, 128, S), np.float16)
        vb = np.zeros((BL, 4, 128, K), np.float16)
        for i in range(BL):
            b = j * BL + i
            kb[i] = keys[:, b, :].T.reshape(4, 128, S).astype(np.float16)
            vpad = np.zeros((512, K), f32)
            vpad[:S] = values[:, b, :]
            vb[i] = vpad.reshape(4, 128, K).astype(np.float16)

        in_maps.append({
            "wctx": chunked(np.ascontiguousarray(wih0_s[:, E:].T), 4, GS).astype(np.float16),
            "whh0": chunked(np.ascontiguousarray(whh0_s.T), 8, GS).astype(np.float16),
            "wih1": chunked(np.ascontiguousarray(shard(inputs["w_ih1"]).T), 8, GS).astype(np.float16),
            "whh1": chunked(np.ascontiguousarray(shard(inputs["w_hh1"]).T), 8, GS).astype(np.float16),
            "wih2": chunked(np.ascontiguousarray(shard(inputs["w_ih2"]).T), 8, GS).astype(np.float16),
            "whh2": chunked(np.ascontiguousarray(shard(inputs["w_hh2"]).T), 8, GS).astype(np.float16),
            "wproj": chunked(np.ascontiguousarray(w_proj.T), 8, K).astype(np.float16),
            "wscore": chunked(np.ascontiguousarray(w_score.T), 8, V),
            "embg": embg.astype(np.float16),
            "bias1": (shard(inputs["b_ih1"]) + shard(inputs["b_hh1"]))[None, :].astype(np.float16),
            "bias2": (shard(inputs["b_ih2"]) + shard(inputs["b_hh2"]))[None, :].astype(np.float16),
            "bproj": b_proj[None, :].astype(np.float16),
            "bscore": b_score[:, None].astype(f32),
            "oh": oh.astype(np.float16),
            "sel": sel,
            "eye64": eye64,
            "eye8": eye8,
            "keys_t": kb,
            "vals_t": vb,
        })
    return in_maps


def kernel(**inputs):
    T = int(np.asarray(inputs["labels"]).shape[1])
    in_maps = _prep_inputs(inputs, T)
    if T not in _cache:
        _cache[T] = _build(T)
    nc = _cache[T]
    res = run_bass_kernel_spmd(nc, in_maps, core_ids=list(range(N_CORES)))
    preds = np.empty((B, T, V), np.float32)
    attn = np.empty((B, T, S), np.float32)
    half = T // 2
    for j in range(N_CORES):
        sc = res.results[j]["scores_out"].reshape(V, 2, half, BL)
        ao = res.results[j]["attn_out"]  # [T, BL, S]
        for i in range(BL):
            b = j * BL + i
            preds[b, :half] = sc[:, 0, :, i].T
            preds[b, half:] = sc[:, 1, :, i].T
            attn[b] = ao[:, i, :]
    return preds, attn


# revision 13
# speedup vs baseline: 1.0803x; 1.0700x over previous
"""Trainium2 Bass kernel for nn_Decoder: 3-layer LSTM decoder with attention.

Strategy: tensor-parallel over the hidden/gate dim across 8 cores (each core
holds a 512-row gate slice of every LSTM cell, fp32, resident in SBUF),
with 4 small AllGathers per timestep (h1,h2,h3,ctx — all sent transposed so
the gathered result is directly usable as matmul lhsT chunks). Attention is
batch-sharded (8 batch elements per core, keys/values bf16 in SBUF). The
final score projection is hoisted out of the time loop and computed as one
batched matmul at the end. Softmax uses sigmoid (exp(x) = s/(1-s)) so the
whole kernel uses one ACT table set.
"""
import numpy as np
import concourse.bass as bass
import concourse.bacc as bacc
import concourse.mybir as mybir
import concourse.tile as tile
from concourse.bass_utils import run_bass_kernel_spmd

N_CORES = 8
B, S, E, H, K, V = 64, 400, 512, 1024, 512, 34
BL = B // N_CORES       # 8 local batch elements (attention)
GS = 4 * H // N_CORES   # 512 gate rows per core
HS = H // N_CORES       # 128 h-units per core
F32 = mybir.dt.float32
F16 = mybir.dt.float16
SIG = mybir.ActivationFunctionType.Sigmoid
TANH = mybir.ActivationFunctionType.Tanh
COPY = mybir.ActivationFunctionType.Copy
AX = mybir.AxisListType.X
MUL = mybir.AluOpType.mult
ADD = mybir.AluOpType.add

_cache = {}


def _build(T):
    nc = bacc.Bacc("TRN2", target_bir_lowering=False, debug=False,
                   enable_asserts=False, num_devices=N_CORES)
    dt = nc.dram_tensor
    # per-core weight shards (transposed, chunked on contract dim)
    wctx_d = dt("wctx", [4, 128, GS], F16, kind="ExternalInput")
    whh0_d = dt("whh0", [8, 128, GS], F16, kind="ExternalInput")
    wih1_d = dt("wih1", [8, 128, GS], F16, kind="ExternalInput")
    whh1_d = dt("whh1", [8, 128, GS], F16, kind="ExternalInput")
    wih2_d = dt("wih2", [8, 128, GS], F16, kind="ExternalInput")
    whh2_d = dt("whh2", [8, 128, GS], F16, kind="ExternalInput")
    wproj_d = dt("wproj", [8, 128, K], F16, kind="ExternalInput")
    wscore_d = dt("wscore", [8, 128, V], F32, kind="ExternalInput")
    embg_d = dt("embg", [V + 1, GS], F16, kind="ExternalInput")
    bias1_d = dt("bias1", [1, GS], F16, kind="ExternalInput")
    bias2_d = dt("bias2", [1, GS], F16, kind="ExternalInput")
    bproj_d = dt("bproj", [1, K], F16, kind="ExternalInput")
    bscore_d = dt("bscore", [V, 1], F32, kind="ExternalInput")
    oh_d = dt("oh", [T, V + 1, B], F16, kind="ExternalInput")
    sel_d = dt("sel", [B, BL], F32, kind="ExternalInput")
    eye64_d = dt("eye64", [B, B], F32, kind="ExternalInput")
    eye8_d = dt("eye8", [BL, BL], F32, kind="ExternalInput")
    keys_d = dt("keys_t", [BL, 4, 128, S], F16, kind="ExternalInput")
    vals_d = dt("vals_t", [BL, 4, 128, K], F16, kind="ExternalInput")
    attn_o = dt("attn_out", [T, BL, S], F32, kind="ExternalOutput")
    score_o = dt("scores_out", [V, T * BL], F32, kind="ExternalOutput")

    with tile.TileContext(nc) as tc:
        with (
            tc.tile_pool(name="const", bufs=1) as cp,
            tc.tile_pool(name="work", bufs=2) as wp,
            tc.tile_pool(name="gath", bufs=2) as gp,
            tc.tile_pool(name="ps", bufs=1, space="PSUM") as pp,
            tc.tile_pool(name="dram", bufs=2, space="DRAM") as dp,
        ):
            # ---- resident tensors ----
            wctx = cp.tile([128, 4, GS], F16)
            nc.sync.dma_start(wctx[:], wctx_d.rearrange("c p g -> p c g"))
            whh0 = cp.tile([128, 8, GS], F16)
            nc.sync.dma_start(whh0[:], whh0_d.rearrange("c p g -> p c g"))
            wih1 = cp.tile([128, 8, GS], F16)
            nc.sync.dma_start(wih1[:], wih1_d.rearrange("c p g -> p c g"))
            whh1 = cp.tile([128, 8, GS], F16)
            nc.sync.dma_start(whh1[:], whh1_d.rearrange("c p g -> p c g"))
            wih2 = cp.tile([128, 8, GS], F16)
            nc.sync.dma_start(wih2[:], wih2_d.rearrange("c p g -> p c g"))
            whh2 = cp.tile([128, 8, GS], F16)
            nc.sync.dma_start(whh2[:], whh2_d.rearrange("c p g -> p c g"))
            wproj = cp.tile([128, 8, K], F16)
            nc.sync.dma_start(wproj[:], wproj_d.rearrange("c p g -> p c g"))
            wscore = cp.tile([128, 8, V], F32)
            nc.sync.dma_start(wscore[:], wscore_d.rearrange("c p g -> p c g"))
            embg = cp.tile([V + 1, GS], F16)
            nc.sync.dma_start(embg[:], embg_d[:])
            bias1 = cp.tile([1, GS], F16)
            nc.sync.dma_start(bias1[:], bias1_d[:])
            bias2 = cp.tile([1, GS], F16)
            nc.sync.dma_start(bias2[:], bias2_d[:])
            bproj = cp.tile([1, K], F16)
            nc.sync.dma_start(bproj[:], bproj_d[:])
            bscore = cp.tile([V, 1], F32)
            nc.sync.dma_start(bscore[:], bscore_d[:])
            sel = cp.tile([B, BL], F32)
            nc.sync.dma_start(sel[:], sel_d[:])
            eye64 = cp.tile([B, B], F32)
            nc.sync.dma_start(eye64[:], eye64_d[:])
            eye8 = cp.tile([BL, BL], F32)
            nc.sync.dma_start(eye8[:], eye8_d[:])
            keyst = cp.tile([128, BL, 4, S], F16)
            nc.sync.dma_start(keyst[:], keys_d.rearrange("b c p s -> p b c s"))
            valst = cp.tile([128, BL, 4, K], F16)
            nc.sync.dma_start(valst[:], vals_d.rearrange("b c p s -> p b c s"))
            ones = cp.tile([1, B], F16)
            nc.vector.memset(ones[:], 1.0)
            zrow = cp.tile([1, GS], F16)
            nc.vector.memset(zrow[:], 0.0)
            # LSTM cell state (updated in place each step)
            c1 = cp.tile([B, HS], F32)
            nc.vector.memset(c1[:], 0.0)
            c2 = cp.tile([B, HS], F32)
            nc.vector.memset(c2[:], 0.0)
            c3 = cp.tile([B, HS], F32)
            nc.vector.memset(c3[:], 0.0)
            # DRAM accumulators for deferred score matmul
            qT_acc = dp.tile([T, 128, 4 * BL], F32, bufs=1)
            cT_acc = dp.tile([T, 128, 4 * BL], F32, bufs=1)

            prev = {"h1T": None, "h2T": None, "h3T": None, "ctxT": None}

            def gather_hT(h, tag):
                """transpose h [64,HS] -> [HS,64], allgather -> [128, 8, 64]."""
                tp_ps = pp.tile([128, B], F32, tag="tp", bufs=2)
                nc.tensor.matmul(tp_ps[:], h[:], eye64[:], start=True, stop=True)
                hT = wp.tile([128, B], F16, tag=f"hT{tag}", bufs=2)
                nc.vector.tensor_copy(hT[:], tp_ps[:])
                bin_ = dp.tile([128, B], F16, tag=f"bi{tag}", name=f"bi{tag}")
                nc.sync.dma_start(bin_[:], hT[:])
                bout = dp.tile([N_CORES, 128, B], F16, addr_space="Shared",
                               tag=f"bo{tag}", name=f"bo{tag}")
                nc.gpsimd.collective_compute(
                    "AllGather", mybir.AluOpType.bypass,
                    replica_groups=[list(range(N_CORES))],
                    ins=[bin_.opt()], outs=[bout.opt()])
                hT_all = gp.tile([128, N_CORES, B], F16, tag=f"ga{tag}")
                nc.sync.dma_start(hT_all[:], bout.rearrange("c p b -> p c b"))
                return hT_all

            def cell_nl(g_ps, c_st, tag):
                """gates psum [64, GS] (i|f|o|g) + state -> h [64, HS]."""
                sfo = wp.tile([B, 3 * HS], F32, tag=f"sfo{tag}")
                nc.scalar.activation(sfo[:], g_ps[:, 0:3 * HS], SIG)
                tg = wp.tile([B, HS], F32, tag=f"tg{tag}")
                nc.scalar.activation(tg[:], g_ps[:, 3 * HS:GS], TANH)
                t1 = wp.tile([B, HS], F32, tag=f"t1{tag}")
                nc.vector.tensor_tensor(t1[:], sfo[:, HS:2 * HS], c_st[:], MUL)
                t2 = wp.tile([B, HS], F32, tag=f"t2{tag}")
                nc.vector.tensor_tensor(t2[:], sfo[:, 0:HS], tg[:], MUL)
                nc.vector.tensor_tensor(c_st[:], t1[:], t2[:], ADD)
                tc_ = wp.tile([B, HS], F32, tag=f"tc{tag}")
                nc.scalar.activation(tc_[:], c_st[:], TANH)
                h = wp.tile([B, HS], F32, tag=f"h{tag}", bufs=2)
                nc.vector.tensor_tensor(h[:], sfo[:, 2 * HS:3 * HS], tc_[:], MUL)
                return h

            def start_g0(t):
                """emb one-hot (+ hh0 when h1[t-1] is gathered) into a fresh
                gates psum — emitted inside the previous AG window so the PE
                stays busy (HAM-warm) during the collective."""
                oh_t = wp.tile([V + 1, B], F16, tag="oh")
                nc.sync.dma_start(oh_t[:], oh_d[t])
                g0 = pp.tile([B, GS], F32, tag="gates", bufs=3, name=f"g0_{t}")
                nc.tensor.matmul(g0[:], oh_t[:], embg[:], start=True, stop=False)
                if prev["h1T"] is not None:
                    for c in range(8):
                        nc.tensor.matmul(g0[:], prev["h1T"][:, c, :],
                                         whh0[:, c, :], start=False, stop=False)
                return g0

            g0 = None
            for t in range(T):
                if g0 is None:
                    g0 = start_g0(0)
                # ---- cell 0: pre-started psum + ctx(t-1) part ----
                if t > 0:
                    for c in range(4):
                        nc.tensor.matmul(g0[:], prev["ctxT"][:, c, :],
                                         wctx[:, c, :], start=False,
                                         stop=(c == 3))
                else:
                    nc.tensor.matmul(g0[:], ones[:], zrow[:],
                                     start=False, stop=True)
                h1 = cell_nl(g0, c1, "a")
                h1T_all = gather_hT(h1, "h1")

                # fill AG(h1) window: bias + hh part of cell 1 (deps ready)
                g1 = pp.tile([B, GS], F32, tag="gates", bufs=3, name=f"g1_{t}")
                nc.tensor.matmul(g1[:], ones[:], bias1[:], start=True, stop=False)
                if t > 0:
                    for c in range(8):
                        nc.tensor.matmul(g1[:], prev["h2T"][:, c, :],
                                         whh1[:, c, :], start=False, stop=False)
                for c in range(8):
                    nc.tensor.matmul(g1[:], h1T_all[:, c, :], wih1[:, c, :],
                                     start=False, stop=(c == 7))
                h2 = cell_nl(g1, c2, "b")
                h2T_all = gather_hT(h2, "h2")

                # fill AG(h2) window: bias + hh part of cell 2
                g2 = pp.tile([B, GS], F32, tag="gates", bufs=3, name=f"g2_{t}")
                nc.tensor.matmul(g2[:], ones[:], bias2[:], start=True, stop=False)
                if t > 0:
                    for c in range(8):
                        nc.tensor.matmul(g2[:], prev["h3T"][:, c, :],
                                         whh2[:, c, :], start=False, stop=False)
                for c in range(8):
                    nc.tensor.matmul(g2[:], h2T_all[:, c, :], wih2[:, c, :],
                                     start=False, stop=(c == 7))
                h3 = cell_nl(g2, c3, "c")
                h3T_all = gather_hT(h3, "h3")

                # fill AG(h3) window: next step's emb + hh0 (h1T just gathered)
                prev = {"h1T": h1T_all, "h2T": h2T_all, "h3T": h3T_all,
                        "ctxT": prev["ctxT"]}
                g0 = start_g0(t + 1) if t + 1 < T else None

                # ---- q = h3 @ w_proj.T + b_proj (replicated) ----
                q_ps = pp.tile([B, K], F32, tag="gates", bufs=3)
                nc.tensor.matmul(q_ps[:], ones[:], bproj[:], start=True, stop=False)
                for c in range(8):
                    nc.tensor.matmul(q_ps[:], h3T_all[:, c, :], wproj[:, c, :],
                                     start=False, stop=(c == 7))
                q_sb = wp.tile([B, K], F32, tag="qsb")
                nc.scalar.activation(q_sb[:], q_ps[:], COPY)
                # select my 8 batches + transpose: qT [128, 4, 8]
                qt_ps = pp.tile([128, 4 * BL], F32, tag="tp", bufs=2)
                for c in range(4):
                    nc.tensor.matmul(qt_ps[:, c * BL:(c + 1) * BL],
                                     q_sb[:, c * 128:(c + 1) * 128], sel[:],
                                     start=True, stop=True)
                qT_bf = wp.tile([128, 4 * BL], F16, tag="qTbf")
                nc.vector.tensor_copy(qT_bf[:], qt_ps[:])
                qT_f = wp.tile([128, 4 * BL], F32, tag="qTf")
                nc.vector.tensor_copy(qT_f[:], qt_ps[:])
                nc.sync.dma_start(qT_acc[t], qT_f[:])

                # ---- attention energies (per-b matvec, col-tiled 4x) ----
                e_ps = [pp.tile([128, S], F32, tag="att", bufs=3, name=f"e{g}_{t}")
                        for g in range(2)]
                for b in range(BL):
                    ps = e_ps[b // 4]
                    row = 32 * (b % 4)
                    for c in range(4):
                        nc.tensor.matmul(
                            ps[row:row + 1, :],
                            qT_bf[:, c * BL + b:c * BL + b + 1],
                            keyst[:, b, c, :],
                            start=(c == 0), stop=(c == 3),
                            tile_position=(0, row))
                # ---- compact energies into [8, S], then sigmoid-softmax ----
                e_sb = wp.tile([128, 2, S], F32, tag="esb", bufs=1)
                for g in range(2):
                    nc.vector.tensor_copy(e_sb[:, g, :], e_ps[g][:])
                e8 = wp.tile([BL, S], F32, tag="e8")
                for g in range(2):
                    ev = e_sb.rearrange("(a b) g s -> a b g s", b=32)[:, 0, g, :]
                    nc.sync.dma_start(e8[g * 4:(g + 1) * 4, :], ev)
                m8 = wp.tile([BL, 1], F32, tag="m8")
                nc.vector.reduce_max(m8[:], e8[:], axis=AX)
                sg8 = wp.tile([BL, S], F32, tag="sg8")
                nc.scalar.activation(sg8[:], e8[:], SIG, bias=m8[:], scale=-1.0)
                u8 = wp.tile([BL, S], F32, tag="u8")
                nc.vector.reciprocal(u8[:], sg8[:])
                attn8 = wp.tile([BL, S], F32, tag="attn8")
                nc.vector.tensor_scalar_add(attn8[:], u8[:], -1.0)  # exp(y)
                z8 = wp.tile([BL, 1], F32, tag="z8")
                nc.vector.reduce_sum(z8[:], attn8[:], axis=AX)
                nc.vector.reciprocal(z8[:], z8[:])
                nc.vector.tensor_scalar_mul(attn8[:], attn8[:], z8[:])
                nc.sync.dma_start(attn_o[t], attn8[:])

                # ---- transpose attn -> attnT [128, 4, 8] (bf16) ----
                at_ps = pp.tile([128, 4 * BL], F32, tag="tp", bufs=2)
                for c in range(4):
                    pr = 128 if c < 3 else S - 3 * 128
                    nc.tensor.matmul(at_ps[0:pr, c * BL:(c + 1) * BL],
                                     attn8[:, c * 128:c * 128 + pr], eye8[:],
                                     start=True, stop=True)
                aT_bf = wp.tile([128, 4 * BL], F16, tag="aTbf")
                nc.vector.memset(aT_bf[:], 0.0)
                nc.vector.tensor_copy(aT_bf[:, 0:3 * BL], at_ps[:, 0:3 * BL])
                nc.vector.tensor_copy(aT_bf[0:S - 3 * 128, 3 * BL:4 * BL],
                                      at_ps[0:S - 3 * 128, 3 * BL:4 * BL])

                # ---- ctx = attn @ V (per-b, col-tiled) ----
                c_ps = [pp.tile([128, K], F32, tag="att", bufs=3, name=f"c{g}_{t}")
                        for g in range(2)]
                for b in range(BL):
                    ps = c_ps[b // 4]
                    row = 32 * (b % 4)
                    for c in range(4):
                        nc.tensor.matmul(
                            ps[row:row + 1, :],
                            aT_bf[:, c * BL + b:c * BL + b + 1],
                            valst[:, b, c, :],
                            start=(c == 0), stop=(c == 3),
                            tile_position=(0, row))
                # compact ctx rows into [8, K]
                c_sb = wp.tile([128, 2, K], F32, tag="csb", bufs=1)
                for g in range(2):
                    nc.vector.tensor_copy(c_sb[:, g, :], c_ps[g][:])
                ctx8 = wp.tile([BL, K], F32, tag="ctx8")
                for g in range(2):
                    cv = c_sb.rearrange("(a b) g s -> a b g s", b=32)[:, 0, g, :]
                    nc.sync.dma_start(ctx8[g * 4:(g + 1) * 4, :], cv)
                # transpose ctx -> ctxT [128, 4, 8]
                ct_ps = pp.tile([128, 4 * BL], F32, tag="tp", bufs=2)
                for c in range(4):
                    nc.tensor.matmul(ct_ps[:, c * BL:(c + 1) * BL],
                                     ctx8[:, c * 128:(c + 1) * 128], eye8[:],
                                     start=True, stop=True)
                cT_f = wp.tile([128, 4 * BL], F32, tag="cTf")
                nc.vector.tensor_copy(cT_f[:], ct_ps[:])
                nc.sync.dma_start(cT_acc[t], cT_f[:])
                cT_h = wp.tile([128, 4 * BL], F16, tag="cTh")
                nc.vector.tensor_copy(cT_h[:], ct_ps[:])
                # allgather ctxT
                cbin = dp.tile([128, 4 * BL], F16, tag="cbi", name=f"cbi_{t}")
                nc.sync.dma_start(cbin[:], cT_h[:])
                cbout = dp.tile([N_CORES, 128, 4 * BL], F16, addr_space="Shared",
                                tag="cbo", name=f"cbo_{t}")
                nc.gpsimd.collective_compute(
                    "AllGather", mybir.AluOpType.bypass,
                    replica_groups=[list(range(N_CORES))],
                    ins=[cbin.opt()], outs=[cbout.opt()])
                ctxT_all = gp.tile([128, 4, N_CORES, BL], F16, tag="gactx")
                nc.sync.dma_start(
                    ctxT_all[:],
                    cbout.rearrange("c p (k b) -> p k c b", b=BL))

                prev["ctxT"] = ctxT_all.rearrange("p k c b -> p k (c b)")

            # ---- deferred scores: [V, T*BL] = wscore.T @ [q;ctx] + b ----
            sc_sb = wp.tile([V, T * BL], F32, tag="scsb", bufs=1)
            for hf in range(2):
                t0, t1_ = hf * (T // 2), (hf + 1) * (T // 2)
                n = (t1_ - t0) * BL
                s_ps = pp.tile([V, n], F32, tag="att", bufs=3, name=f"sps{hf}")
                pairs = [(si, c) for si in range(2) for c in range(4)]
                for idx, (si, c) in enumerate(pairs):
                    src = (qT_acc, cT_acc)[si]
                    rhs_t = wp.tile([128, t1_ - t0, BL], F32, tag="srhs")
                    nc.sync.dma_start(
                        rhs_t[:],
                        src[t0:t1_].rearrange("t p (k b) -> p k t b", b=BL)
                        [:, c, :, :])
                    nc.tensor.matmul(s_ps[:], wscore[:, c + 4 * si, :],
                                     rhs_t[:], start=(idx == 0),
                                     stop=(idx == len(pairs) - 1))
                nc.scalar.activation(sc_sb[:, t0 * BL:t1_ * BL], s_ps[:],
                                     mybir.ActivationFunctionType.Identity,
                                     bias=bscore[:])
            nc.sync.dma_start(score_o[:], sc_sb[:])

    nc.compile()
    return nc


def _prep_inputs(inputs, T):
    """Build per-core in_maps from full inputs."""
    f32 = np.float32
    labels = np.asarray(inputs["labels"]).astype(np.int64)
    keys = np.asarray(inputs["keys"], f32)
    values = np.asarray(inputs["values"], f32)
    emb = np.asarray(inputs["emb"], f32)
    w_proj = np.asarray(inputs["w_proj"], f32)
    b_proj = np.asarray(inputs["b_proj"], f32)
    w_score = np.asarray(inputs["w_score"], f32)
    b_score = np.asarray(inputs["b_score"], f32)
    h0 = np.asarray(inputs["h0"], f32)

    eye64 = np.eye(B, dtype=f32)
    eye8 = np.eye(BL, dtype=f32)
    oh = np.zeros((T, V + 1, B), f32)
    for t in range(T):
        oh[t, labels[:, t], np.arange(B)] = 1.0
    oh[0, V, :] = 1.0  # initial-context constant row

    # torch gate order i,f,g,o ; our section order i,f,o,g
    blk = {"i": 0, "f": 1, "g": 2, "o": 3}
    in_maps = []
    ctx0 = (h0[0] @ w_proj.T + b_proj).astype(f32)  # [K], h0 rows identical
    for j in range(N_CORES):
        rows = np.concatenate([
            np.arange(blk[g] * H + j * HS, blk[g] * H + (j + 1) * HS)
            for g in ("i", "f", "o", "g")])

        def shard(w):
            return np.ascontiguousarray(np.asarray(w, f32)[rows])

        wih0_s = shard(inputs["w_ih0"])      # [GS, E+K]
        whh0_s = shard(inputs["w_hh0"])      # [GS, H]
        b0 = shard(inputs["b_ih0"]) + shard(inputs["b_hh0"])
        embg = np.concatenate([
            emb @ wih0_s[:, :E].T + b0[None, :],
            (ctx0 @ wih0_s[:, E:].T + 0.0)[None, :]], axis=0).astype(f32)

        def chunked(wT, nch, width):
            return np.ascontiguousarray(wT.reshape(nch, 128, width))

        sel = np.zeros((B, BL), f32)
        sel[np.arange(j * BL, (j + 1) * BL), np.arange(BL)] = 1.0

        kb = np.zeros((BL, 4, 128, S), np.float16)
        vb = np.zeros((BL, 4, 128, K), np.float16)
        for i in range(BL):
            b = j * BL + i
            kb[i] = keys[:, b, :].T.reshape(4, 128, S).astype(np.float16)
            vpad = np.zeros((512, K), f32)
            vpad[:S] = values[:, b, :]
            vb[i] = vpad.reshape(4, 128, K).astype(np.float16)

        in_maps.append({
            "wctx": chunked(np.ascontiguousarray(wih0_s[:, E:].T), 4, GS).astype(np.float16),
            "whh0": chunked(np.ascontiguousarray(whh0_s.T), 8, GS).astype(np.float16),
            "wih1": chunked(np.ascontiguousarray(shard(inputs["w_ih1"]).T), 8, GS).astype(np.float16),
            "whh1": chunked(np.ascontiguousarray(shard(inputs["w_hh1"]).T), 8, GS).astype(np.float16),
            "wih2": chunked(np.ascontiguousarray(shard(inputs["w_ih2"]).T), 8, GS).astype(np.float16),
            "whh2": chunked(np.ascontiguousarray(shard(inputs["w_hh2"]).T), 8, GS).astype(np.float16),
            "wproj": chunked(np.ascontiguousarray(w_proj.T), 8, K).astype(np.float16),
            "wscore": chunked(np.ascontiguousarray(w_score.T), 8, V),
            "embg": embg.astype(np.float16),
            "bias1": (shard(inputs["b_ih1"]) + shard(inputs["b_hh1"]))[None, :].astype(np.float16),
            "bias2": (shard(inputs["b_ih2"]) + shard(inputs["b_hh2"]))[None, :].astype(np.float16),
            "bproj": b_proj[None, :].astype(np.float16),
            "bscore": b_score[:, None].astype(f32),
            "oh": oh.astype(np.float16),
            "sel": sel,
            "eye64": eye64,
            "eye8": eye8,
            "keys_t": kb,
            "vals_t": vb,
        })
    return in_maps


def kernel(**inputs):
    T = int(np.asarray(inputs["labels"]).shape[1])
    in_maps = _prep_inputs(inputs, T)
    if T not in _cache:
        _cache[T] = _build(T)
    nc = _cache[T]
    res = run_bass_kernel_spmd(nc, in_maps, core_ids=list(range(N_CORES)))
    preds = np.empty((B, T, V), np.float32)
    attn = np.empty((B, T, S), np.float32)
    half = T // 2
    for j in range(N_CORES):
        sc = res.results[j]["scores_out"].reshape(V, 2, half, BL)
        ao = res.results[j]["attn_out"]  # [T, BL, S]
        for i in range(BL):
            b = j * BL + i
            preds[b, :half] = sc[:, 0, :, i].T
            preds[b, half:] = sc[:, 1, :, i].T
            attn[b] = ao[:, i, :]
    return preds, attn
